# revision 1
# baseline (speedup 1.0000x reference)
"""Trainium2 Bass kernel for AdjAttenAgger-style masked cross-attention.

Computes, for full inputs:
    Q = main_feat @ Wq.T + bq              # [N, MID]
    K = other_feat @ Wk.T + bk             # [M, MID]
    attn = softmax(where(mask, -BIG, Q K^T / sqrt(MID)), axis=-1)
    out  = attn @ (fix_feat[:, None] * other_feat)          # [N, KDIM]

Sharding: rows of main_feat/mask (the N query axis) are split across 8
NeuronCores; other_feat/fix_feat/weights are replicated. No collectives.

Per-core dataflow (all layouts chosen so no large tensor is ever
transposed outside the PE array):
  - QT [MID, nq] and KT [MID, nk] are built dim-major via PE-transposed
    input tiles, so the QK^T matmul directly produces attnT [k, q] slabs.
  - The boolean mask (q-major in DRAM, only efficiently loadable q-major)
    is applied *by the PE*: an accumulating matmul with the q-major mask
    tile as the stationary operand and a scaled diagonal as the moving
    operand adds -BIG * mask^T into the attnT PSUM tile.
  - ACT computes exp((attnT - BIG*mask)/sqrt(MID)) PSUM->SBUF; no row-max
    subtraction is needed (logits are O(1); masked entries underflow to 0).
  - V' = [fix*other | 1] has an extra ones column, so the attn@V' matmul
    also produces the softmax denominators; a per-row divide finishes the
    softmax normalization on the [nq, 256] output only.

Matmul operands use float32r (full-rate fp32 streaming) when the moving
free dim is >= 256; the mask matmul uses bf16/fp8 operands (exact for
values {0, -2^41} / {0, 2^-9}).
"""

import math

import numpy as np

import concourse.bass as bass
from concourse import bacc
import concourse.mybir as mybir
import concourse.tile as tile
from concourse.bass_utils import run_bass_kernel_spmd

F32 = mybir.dt.float32
F32R = mybir.dt.float32r
BF16 = mybir.dt.bfloat16
U8 = mybir.dt.uint8
U16 = mybir.dt.uint16
F8E4 = mybir.dt.float8e4

N_CORES = 8
QDIM = 256       # main/other feature dim
MID = 128
NEG_BIG = -float(2 ** 41)  # additive pre-scale mask value; exp() underflows to 0
F8_SUB = 2.0 ** -9         # value of byte 0x01 reinterpreted as float8e4 (e4m3)
F8_MAX = 240.0             # fp8e4 (IEEE e4m3) max normal
EPS_DR = 2.0 ** -12        # Q prescale for fp8x8 DoubleRow mask (product -0.875)


def _diag(nc, ap, fill):
    """ap[i, j] = fill if i == j else 0."""
    nc.gpsimd.memset(ap, 0.0)
    nc.gpsimd.affine_select(
        out=ap, in_=ap,
        compare_op=mybir.AluOpType.not_equal,
        fill=fill, base=0,
        pattern=[[-1, ap.shape[1]]],
        channel_multiplier=1,
    )


def declare_io(nc, nq, nkeys):
    return {
        "main": nc.dram_tensor("main", [nq, QDIM], F32, kind="ExternalInput").ap(),
        "mask": nc.dram_tensor("mask", [nq, nkeys], U8, kind="ExternalInput").ap(),
        "other": nc.dram_tensor("other", [nkeys, QDIM], F32, kind="ExternalInput").ap(),
        "fix": nc.dram_tensor("fix", [nkeys, 1], F32, kind="ExternalInput").ap(),
        "Wq": nc.dram_tensor("Wq", [MID, QDIM], F32, kind="ExternalInput").ap(),
        "bq": nc.dram_tensor("bq", [MID, 1], F32, kind="ExternalInput").ap(),
        "Wk": nc.dram_tensor("Wk", [MID, QDIM], F32, kind="ExternalInput").ap(),
        "bk": nc.dram_tensor("bk", [MID, 1], F32, kind="ExternalInput").ap(),
        "out": nc.dram_tensor("out", [nq, QDIM], F32, kind="ExternalOutput").ap(),
    }


def emit_kernel(tc, nq, nkeys, q_group=512, mm_dt=F32R, mask_mode="fp8",
                io=None):
    """Emit the per-core program. nq = queries this core, nkeys = all keys."""
    nc = tc.nc
    n_qt = nq // 128          # query 128-tiles
    n_kt = nkeys // 128       # key 128-tiles
    qg = min(q_group, nq)     # q columns per PSUM slab
    n_qg = nq // qg
    n_qc = qg // 128          # 128-chunks per q group
    inv_sqrt_mid = 1.0 / math.sqrt(MID)
    vw = QDIM + 2             # V' width: 256 dims + ones col + pad (even for f32r)

    if io is None:
        io = declare_io(nc, nq, nkeys)
    main, maskd, other, fix = io["main"], io["mask"], io["other"], io["fix"]
    wq, bq, wk, bk, out = io["Wq"], io["bq"], io["Wk"], io["bk"], io["out"]

    # mask viewed as [qg-group, kt-group, partition(q), qc-chunk, k]
    KTG = min(16, n_kt)    # key tiles per mask DMA (2KB contiguous chunks)
    mask_rg = maskd.rearrange(
        "(qh qc p) (ktg k) -> qh ktg p qc k", qc=n_qc, p=128, k=KTG * 128
    )
    n_qc2 = max(1, qg // 256)  # 256-row chunks for DoubleRow mask MMs
    mask_dr = maskd.rearrange(
        "(qh qc2 p j) (ktg k) -> qh ktg p qc2 j k",
        qc2=n_qc2, p=qg // (2 * n_qc2), j=2, k=KTG * 128,
    )
    # For dma_t mode: mask as u16 words [qh, wt, q, w]; keys permuted
    # globally as k' = s*(nkeys/2) + w  <->  original key 2w+s.
    n_half = nkeys // 2
    mask_u16_r = maskd.bitcast(U16).rearrange(
        "(qh p) (wt w) -> qh wt p w", p=qg, w=128
    )
    other_sw = other.rearrange("(w s) d -> s w d", s=2)
    fix_sw = fix.rearrange("(w s) d -> s w d", s=2)
    permute_keys = mask_mode in ("dma_t", "dmat_pe")

    def other_block(p2):
        """[128, 2, QDIM] view of rows for k'-tile pair p2 (256 keys)."""
        if not permute_keys:
            return (other[p2 * 256 : (p2 + 1) * 256, :]
                    .rearrange("(a p) d -> p a d", p=128))
        s, w0 = divmod(p2 * 256, n_half)
        return (other_sw[s, w0 : w0 + 256, :]
                .rearrange("(a p) d -> p a d", p=128))

    def fix_block(p2):
        if not permute_keys:
            return (fix[p2 * 256 : (p2 + 1) * 256, :]
                    .rearrange("(a p) d -> p a d", p=128))
        s, w0 = divmod(p2 * 256, n_half)
        return (fix_sw[s, w0 : w0 + 256, :]
                .rearrange("(a p) d -> p a d", p=128))

    ident = mybir.ActivationFunctionType.Identity
    expf = mybir.ActivationFunctionType.Exp

    with (
        tc.tile_pool(name="const", bufs=1) as constp,
        tc.tile_pool(name="big", bufs=1) as bigp,
    ):
        # ---- constants ----
        ident_f32 = constp.tile([128, 128], F32)
        _diag(nc, ident_f32, 1.0)
        if mm_dt == F32:
            ident_t = ident_f32
        else:
            ident_t = constp.tile([128, 128], mm_dt)
            nc.vector.tensor_copy(ident_t, ident_f32)
        if mask_mode == "fp8":
            diag_mm = constp.tile([128, 128], BF16)
            _diag(nc, diag_mm, NEG_BIG * F8_SUB)  # f8 byte 0x01 -> 2^-9
        elif mask_mode == "fp8dr":
            diag_dr = constp.tile([128, 2, 256], F8E4)
            nc.gpsimd.memset(diag_dr, 0.0)
            # fill where 2*ki + j - q' == 0
            nc.gpsimd.affine_select(
                out=diag_dr, in_=diag_dr,
                compare_op=mybir.AluOpType.not_equal,
                fill=-F8_MAX, base=0,
                pattern=[[1, 2], [-1, 256]],
                channel_multiplier=2,
            )
        elif mask_mode == "dmat_pe":
            diag_mm = constp.tile([128, 128], BF16)
            _diag(nc, diag_mm, 1.0)
        else:
            diag_mm = constp.tile([128, 128], BF16)
            _diag(nc, diag_mm, 1.0)

        bq_s = constp.tile([MID, 1], F32)
        nc.sync.dma_start(bq_s, bq)
        if mask_mode == "fp8dr":
            bq_eps = constp.tile([MID, 1], F32)
            nc.vector.tensor_scalar_mul(bq_eps, bq_s, EPS_DR)
            q_bias, q_scale = bq_eps, EPS_DR
            exp_scale = inv_sqrt_mid / EPS_DR
        else:
            q_bias, q_scale = bq_s, 1.0
            exp_scale = inv_sqrt_mid
        bk_s = constp.tile([MID, 1], F32)
        nc.sync.dma_start(bk_s, bk)

        wq_s = constp.tile([MID, QDIM], mm_dt)
        nc.sync.dma_start(wq_s, wq.bitcast(mm_dt))
        wk_s = constp.tile([MID, QDIM], mm_dt)
        nc.sync.dma_start(wk_s, wk.bitcast(mm_dt))

        # ---- persistent big tensors ----
        kt_sb = bigp.tile([MID, nkeys], mm_dt)      # K^T, dim-major
        qt_sb = bigp.tile([MID, nq], mm_dt)         # Q^T, dim-major
        vp_sb = bigp.tile([128, n_kt, vw], mm_dt)   # V' tiles, token-major
        nc.scalar.activation(vp_sb[:, :, QDIM : QDIM + 2], vp_sb[:, :, 0:2],
                             mybir.ActivationFunctionType.Copy,
                             bias=1.0, scale=0.0)

        with (
            tc.tile_pool(name="prologue", bufs=3) as prop,
            tc.tile_pool(name="ppsum", bufs=2, space="PSUM") as ppsum,
        ):
            # WqT / WkT: [qdim-part, h, mid]
            wqt_s = constp.tile([128, 2, MID], mm_dt)
            wkt_s = constp.tile([128, 2, MID], mm_dt)
            for h in range(2):
                wq_ps = ppsum.tile([128, MID], mm_dt, tag="tps", name="wq_ps")
                nc.tensor.transpose(wq_ps, wq_s[:, h * 128 : (h + 1) * 128], ident_t)
                nc.vector.tensor_copy(wqt_s[:, h, :], wq_ps)
                wk_ps = ppsum.tile([128, MID], mm_dt, tag="tps", name="wk_ps")
                nc.tensor.transpose(wk_ps, wk_s[:, h * 128 : (h + 1) * 128], ident_t)
                nc.vector.tensor_copy(wkt_s[:, h, :], wk_ps)

            # ---- Q^T = Wq @ main^T + bq  (pairs of 128-tiles: 256 moving) ----
            for t2 in range(n_qt // 2):
                main_t = prop.tile([128, 2, QDIM], mm_dt)
                nc.sync.dma_start(
                    main_t, main[t2 * 256 : (t2 + 1) * 256, :]
                    .rearrange("(a p) d -> p a d", p=128).bitcast(mm_dt)
                )
                maint_s = prop.tile([128, 2, 2, 128], mm_dt)  # [d-half, h, a, tok]
                for h in range(2):
                    for a in range(2):
                        tp = ppsum.tile([128, 128], mm_dt, tag="tps", name="tp")
                        nc.tensor.transpose(
                            tp, main_t[:, a, h * 128 : (h + 1) * 128], ident_t
                        )
                        nc.vector.tensor_copy(maint_s[:, h, a, :], tp)
                q_ps = ppsum.tile([MID, 256], F32, tag="mps", name="q_ps")
                for h in range(2):
                    nc.tensor.matmul(
                        q_ps,
                        wqt_s[:, h, :],
                        maint_s[:, h, :, :],
                        start=(h == 0),
                        stop=(h == 1),
                    )
                nc.scalar.activation(
                    qt_sb[:, t2 * 256 : (t2 + 1) * 256], q_ps, ident,
                    bias=q_bias, scale=q_scale,
                )

            # ---- K^T = Wk @ other^T + bk ;  V' = [fix*other | 1] ----
            for k2 in range(n_kt // 2):
                other_t = prop.tile([128, 2, QDIM], mm_dt)
                nc.sync.dma_start(other_t, other_block(k2).bitcast(mm_dt))
                fix_t = prop.tile([128, 2, 1], F32)
                nc.sync.dma_start(fix_t, fix_block(k2))
                for a in range(2):
                    nc.gpsimd.tensor_scalar_mul(
                        vp_sb[:, 2 * k2 + a, 0:QDIM],
                        other_t[:, a, :],
                        fix_t[:, a, :],
                    )
                ot_s = prop.tile([128, 2, 2, 128], mm_dt)  # [d-half, h, a, tok]
                for h in range(2):
                    for a in range(2):
                        to = ppsum.tile([128, 128], mm_dt, tag="tps", name="to")
                        nc.tensor.transpose(
                            to, other_t[:, a, h * 128 : (h + 1) * 128], ident_t
                        )
                        nc.vector.tensor_copy(ot_s[:, h, a, :], to)
                k_ps = ppsum.tile([MID, 256], F32, tag="mps", name="k_ps")
                for h in range(2):
                    nc.tensor.matmul(
                        k_ps,
                        wkt_s[:, h, :],
                        ot_s[:, h, :, :],
                        start=(h == 0),
                        stop=(h == 1),
                    )
                nc.scalar.activation(
                    kt_sb[:, k2 * 256 : (k2 + 1) * 256], k_ps, ident, bias=bk_s
                )

        # ---- main attention loop ----
        with (
            tc.tile_pool(name="mwork", bufs=4) as mwork,
            tc.tile_pool(name="apsum", bufs=3, space="PSUM") as apsum,
            tc.tile_pool(name="avpsum", bufs=1, space="PSUM") as avpsum,
            tc.tile_pool(name="outp", bufs=3) as outp,
        ):
            for qh in range(n_qg):
                av_ps = [
                    avpsum.tile([128, vw], F32, tag=f"av{qc}", name=f"av{qc}")
                    for qc in range(n_qc)
                ]
                if mask_mode in ("dma_t", "dmat_pe"):
                    for wt in range(n_kt // 2):
                        mtile = mwork.tile([128, qg], U16, name="mtile")
                        nc.sync.dma_start(
                            mtile, mask_u16_r[qh, wt], transpose=True
                        )
                        m8 = mtile.bitcast(U8).rearrange("p (q s) -> p q s", s=2)
                        for s in range(2):
                            kt = s * (n_kt // 2) + wt
                            attn_ps = apsum.tile([128, qg], F32, name="attn_ps")
                            if mask_mode == "dmat_pe":
                                mask_big = mwork.tile(
                                    [128, qg], BF16, name="mask_big")
                                nc.gpsimd.tensor_scalar(
                                    mask_big, m8[:, :, s], NEG_BIG, None,
                                    mybir.AluOpType.mult,
                                )
                                nc.tensor.matmul(
                                    attn_ps,
                                    kt_sb[:, kt * 128 : (kt + 1) * 128],
                                    qt_sb[:, qh * qg : (qh + 1) * qg],
                                    start=True,
                                    stop=False,
                                )
                                nc.tensor.matmul(
                                    attn_ps,
                                    diag_mm,
                                    mask_big,
                                    start=False,
                                    stop=True,
                                )
                            else:
                                nc.tensor.matmul(
                                    attn_ps,
                                    kt_sb[:, kt * 128 : (kt + 1) * 128],
                                    qt_sb[:, qh * qg : (qh + 1) * qg],
                                    start=True,
                                    stop=True,
                                )
                                nc.vector.scalar_tensor_tensor(
                                    attn_ps, m8[:, :, s], NEG_BIG, attn_ps,
                                    mybir.AluOpType.mult, mybir.AluOpType.add,
                                )
                            expattn = mwork.tile([128, qg], mm_dt, name="expattn")
                            nc.scalar.activation(
                                expattn, attn_ps, expf, scale=exp_scale
                            )
                            for qc in range(n_qc):
                                nc.tensor.matmul(
                                    av_ps[qc],
                                    expattn[:, qc * 128 : (qc + 1) * 128],
                                    vp_sb[:, kt, :],
                                    start=(wt == 0 and s == 0),
                                    stop=(wt == n_kt // 2 - 1 and s == 1),
                                )
                    for qc in range(n_qc):
                        denom = outp.tile([128, 1], F32, name="denom")
                        nc.scalar.copy(denom, av_ps[qc][:, QDIM : QDIM + 1])
                        recip = outp.tile([128, 1], F32, name="recip")
                        scratch = outp.tile([128, 1], F32, name="scratch")
                        nc.vector.reciprocal_approx_accurate(recip, denom, scratch)
                        out_t = outp.tile([128, QDIM], F32, name="out_t")
                        nc.vector.tensor_scalar_mul(
                            out_t, av_ps[qc][:, 0:QDIM], recip)
                        r0 = qh * qg + qc * 128
                        nc.sync.dma_start(out[r0 : r0 + 128, :], out_t)
                    continue
                for ktg in range(n_kt // KTG):
                  mask_gf8 = mwork.tile([128, n_qc, KTG * 128], F8E4,
                                        name="mask_gf8")
                  if mask_mode == "fp8dr":
                    mask_gdr = mwork.tile([128, n_qc2, 2, KTG * 128], F8E4,
                                          name="mask_gdr")
                    for qc2 in range(n_qc2):
                        nc.sync.dma_start(
                            mask_gdr[:, qc2],
                            mask_dr[qh, ktg][:, qc2].bitcast(F8E4))
                  elif mask_mode == "fp8":
                    nc.sync.dma_start(mask_gf8, mask_rg[qh, ktg].bitcast(F8E4))
                  else:
                    mask_gu8 = mwork.tile([128, n_qc, KTG * 128], U8,
                                          name="mask_gu8")
                    nc.sync.dma_start(mask_gu8, mask_rg[qh, ktg])
                    mask_gbf = mwork.tile([128, n_qc, KTG * 128], BF16,
                                          name="mask_gbf")
                    nc.gpsimd.tensor_scalar(
                        mask_gbf, mask_gu8, NEG_BIG, None, mybir.AluOpType.mult
                    )
                  for kti in range(KTG):
                    kt = ktg * KTG + kti
                    attn_ps = apsum.tile([128, qg], F32)
                    if mask_mode == "fp8dr":
                        for qc2 in range(n_qc2):
                            nc.tensor.matmul(
                                attn_ps[:, qc2 * 256 : (qc2 + 1) * 256],
                                mask_gdr[:, qc2, :, kti * 128 : (kti + 1) * 128],
                                diag_dr,
                                start=(qc2 == 0),
                                stop=False,
                                perf_mode=mybir.MatmulPerfMode.DoubleRow,
                                skip_group_check=True,
                            )
                    else:
                        if mask_mode == "fp8":
                            mask_op = mask_gf8[:, :, kti * 128 : (kti + 1) * 128]
                        else:
                            mask_op = mask_gbf[:, :, kti * 128 : (kti + 1) * 128]
                        for qc in range(n_qc):
                            nc.tensor.matmul(
                                attn_ps[:, qc * 128 : (qc + 1) * 128],
                                mask_op[:, qc, :],
                                diag_mm,
                                start=(qc == 0),
                                stop=False,
                                skip_group_check=True,
                            )
                    nc.tensor.matmul(
                        attn_ps,
                        kt_sb[:, kt * 128 : (kt + 1) * 128],
                        qt_sb[:, qh * qg : (qh + 1) * qg],
                        start=False,
                        stop=True,
                        skip_group_check=True,
                    )
                    expattn = mwork.tile([128, qg], mm_dt)
                    nc.scalar.activation(expattn, attn_ps, expf, scale=exp_scale)
                    for qc in range(n_qc):
                        nc.tensor.matmul(
                            av_ps[qc],
                            expattn[:, qc * 128 : (qc + 1) * 128],
                            vp_sb[:, kt, :],
                            start=(kt == 0),
                            stop=(kt == n_kt - 1),
                        )
                for qc in range(n_qc):
                    denom = outp.tile([128, 1], F32)
                    nc.scalar.copy(denom, av_ps[qc][:, QDIM : QDIM + 1])
                    recip = outp.tile([128, 1], F32)
                    scratch = outp.tile([128, 1], F32)
                    nc.vector.reciprocal_approx_accurate(recip, denom, scratch)
                    out_t = outp.tile([128, QDIM], F32)
                    nc.vector.tensor_scalar_mul(
                        out_t, av_ps[qc][:, 0:QDIM], recip)
                    r0 = qh * qg + qc * 128
                    nc.sync.dma_start(out[r0 : r0 + 128, :], out_t)


def build_nc(nq, nkeys, q_group=512, mm_dt=F32R, mask_mode="fp8", repeat=1):
    nc = bacc.Bacc("TRN2", target_bir_lowering=False, debug=False,
                   enable_asserts=False)
    io = declare_io(nc, nq, nkeys)
    with tile.TileContext(nc) as tc:
        for _ in range(repeat):
            emit_kernel(tc, nq, nkeys, q_group=q_group, mm_dt=mm_dt,
                        mask_mode=mask_mode, io=io)
    nc.compile()
    return nc


def make_in_maps(inputs, n_cores=N_CORES):
    """Shard full inputs into per-core input maps."""
    main_feat = np.ascontiguousarray(np.asarray(inputs["main_feat"], dtype=np.float32))
    other_feat = np.ascontiguousarray(np.asarray(inputs["other_feat"], dtype=np.float32))
    fix_feat = np.ascontiguousarray(
        np.asarray(inputs["fix_feat"], dtype=np.float32).reshape(-1, 1)
    )
    mask = np.ascontiguousarray(np.asarray(inputs["mask"])).view(np.uint8)
    wq_ = np.ascontiguousarray(np.asarray(inputs["Wq"], dtype=np.float32))
    bq_ = np.ascontiguousarray(np.asarray(inputs["bq"], dtype=np.float32).reshape(-1, 1))
    wk_ = np.ascontiguousarray(np.asarray(inputs["Wk"], dtype=np.float32))
    bk_ = np.ascontiguousarray(np.asarray(inputs["bk"], dtype=np.float32).reshape(-1, 1))

    n = main_feat.shape[0]
    per = n // n_cores
    in_maps = []
    for c in range(n_cores):
        sl = slice(c * per, (c + 1) * per)
        in_maps.append(
            {
                "main": np.ascontiguousarray(main_feat[sl]),
                "mask": np.ascontiguousarray(mask[sl]),
                "other": other_feat,
                "fix": fix_feat,
                "Wq": wq_,
                "bq": bq_,
                "Wk": wk_,
                "bk": bk_,
            }
        )
    return in_maps


_NC_CACHE = {}


def _get_nc(nq, nkeys):
    key = (nq, nkeys)
    if key not in _NC_CACHE:
        _NC_CACHE[key] = build_nc(nq, nkeys)
    return _NC_CACHE[key]


class _Executor:
    """Cached jit(shard_map) wrapper around the compiled Bass module so
    repeated kernel() calls skip retracing/recompiling."""

    def __init__(self, nc, n_cores=N_CORES):
        import jax
        from jax.sharding import Mesh, PartitionSpec
        from jax.experimental.shard_map import shard_map
        from concourse import bass2jax
        from concourse.bass2jax import _bass_exec_p, install_neuronx_cc_hook

        install_neuronx_cc_hook()
        self.n_cores = n_cores
        partition_name = (
            nc.partition_id_tensor.name if nc.partition_id_tensor else None
        )
        in_names, out_names, out_avals = [], [], []
        for alloc in nc.m.functions[0].allocations:
            if not isinstance(alloc, mybir.MemoryLocationSet):
                continue
            name = alloc.memorylocations[0].name
            if alloc.kind == "ExternalInput":
                if name != partition_name:
                    in_names.append(name)
            elif alloc.kind == "ExternalOutput":
                out_names.append(name)
                out_avals.append(
                    jax.core.ShapedArray(
                        tuple(alloc.tensor_shape), mybir.dt.np(alloc.dtype)
                    )
                )
        self.in_names = list(in_names)
        self.out_names = out_names
        self.out_avals = out_avals
        all_names = in_names + out_names
        if partition_name is not None:
            all_names.append(partition_name)

        def _body(*args):
            operands = list(args)
            if partition_name is not None:
                operands.append(bass2jax.partition_id_tensor())
            return tuple(
                _bass_exec_p.bind(
                    *operands,
                    out_avals=tuple(out_avals),
                    in_names=tuple(all_names),
                    out_names=tuple(out_names),
                    lowering_input_output_aliases=(),
                    sim_require_finite=True,
                    sim_require_nnan=True,
                    nc=nc,
                )
            )

        devices = jax.devices()[:n_cores]
        self.mesh = Mesh(np.asarray(devices), ("core",))
        n_args = len(self.in_names) + len(out_names)
        self.f = jax.jit(
            shard_map(
                _body,
                mesh=self.mesh,
                in_specs=(PartitionSpec("core"),) * n_args,
                out_specs=(PartitionSpec("core"),) * len(out_names),
                check_rep=False,
            ),
            keep_unused=True,
        )

    def run(self, in_maps):
        concat_in = [
            np.concatenate([m[nm] for m in in_maps], axis=0)
            for nm in self.in_names
        ]
        concat_zeros = [
            np.zeros((self.n_cores * a.shape[0], *a.shape[1:]), a.dtype)
            for a in self.out_avals
        ]
        r = self.f(*concat_in, *concat_zeros)
        return np.asarray(r[0])


_EXEC_CACHE = {}


def _get_executor(nq, nkeys):
    key = (nq, nkeys)
    if key not in _EXEC_CACHE:
        _EXEC_CACHE[key] = _Executor(_get_nc(nq, nkeys))
    return _EXEC_CACHE[key]


def kernel(**inputs) -> np.ndarray:
    n = np.asarray(inputs["main_feat"]).shape[0]
    nkeys = np.asarray(inputs["other_feat"]).shape[0]
    in_maps = make_in_maps(inputs, N_CORES)
    try:
        ex = _get_executor(n // N_CORES, nkeys)
        return ex.run(in_maps)
    except Exception:
        nc = _get_nc(n // N_CORES, nkeys)
        res = run_bass_kernel_spmd(nc, in_maps, core_ids=list(range(N_CORES)))
        return np.concatenate(
            [res.results[c]["out"] for c in range(N_CORES)], axis=0
        )



# revision 17
# speedup vs baseline: 1.3078x; 1.3078x over previous
"""Trainium2 Bass kernel for AdjAttenAgger-style masked cross-attention.

Computes, for full inputs:
    Q = main_feat @ Wq.T + bq              # [N, MID]
    K = other_feat @ Wk.T + bk             # [M, MID]
    attn = softmax(where(mask, -BIG, Q K^T / sqrt(MID)), axis=-1)
    out  = attn @ (fix_feat[:, None] * other_feat)          # [N, KDIM]

Sharding: rows of main_feat/mask (the N query axis) are split across 8
NeuronCores; other_feat/fix_feat/weights are replicated. No collectives.

Per-core dataflow (all layouts chosen so no large tensor is ever
transposed outside the PE array):
  - QT [MID, nq] and KT [MID, nk] are built dim-major via PE-transposed
    input tiles, so the QK^T matmul directly produces attnT [k, q] slabs.
  - The boolean mask (q-major in DRAM, only efficiently loadable q-major)
    is applied *by the PE*: accumulating matmuls with the q-major mask
    tile as the stationary operand and a scaled diagonal as the moving
    operand add -BIG * mask^T into the attnT PSUM tile.
  - ACT computes exp((attnT - BIG*mask)/sqrt(MID)) PSUM->SBUF; no row-max
    subtraction is needed (logits are O(1); masked entries underflow to 0).
  - V' = [fix*other | 1] has an extra ones column, so the attn@V' matmul
    also produces the softmax denominators; a per-row divide finishes the
    softmax normalization on the [nq, 256] output only.

Scheduling: the prologue batches the four PE transposes of each 256-row
input chunk into a single PSUM bank evacuated by one DVE copy, and the
main loop emits the mask/QK matmuls of k-tile kt+2 before the AV matmuls
of k-tile kt, so the PE never stalls on the exp activation.
"""

import math
import os

import numpy as np

import concourse.bass as bass
from concourse import bacc
import concourse.mybir as mybir
import concourse.tile as tile
from concourse.bass_utils import run_bass_kernel_spmd

F32 = mybir.dt.float32
F32R = mybir.dt.float32r
BF16 = mybir.dt.bfloat16
U8 = mybir.dt.uint8
F8E4 = mybir.dt.float8e4

N_CORES = 8
QDIM = 256       # main/other feature dim
MID = 128
NEG_BIG = -float(2 ** 41)  # additive pre-scale mask value; exp() underflows to 0
F8_SUB = 2.0 ** -9         # value of byte 0x01 reinterpreted as float8e4 (e4m3)
F8_MAX = 240.0             # fp8e4 (IEEE e4m3) max normal
EPS_DR = 2.0 ** -12        # Q prescale for fp8x8 DoubleRow mask (product -0.875)


def _diag(nc, ap, fill):
    """ap[i, j] = fill if i == j else 0."""
    nc.gpsimd.memset(ap, 0.0)
    nc.gpsimd.affine_select(
        out=ap, in_=ap,
        compare_op=mybir.AluOpType.not_equal,
        fill=fill, base=0,
        pattern=[[-1, ap.shape[1]]],
        channel_multiplier=1,
    )


def declare_io(nc, nq, nkeys):
    return {
        "main": nc.dram_tensor("main", [nq, QDIM], F32, kind="ExternalInput").ap(),
        "mask": nc.dram_tensor("mask", [nq, nkeys], U8, kind="ExternalInput").ap(),
        "other": nc.dram_tensor("other", [nkeys, QDIM], F32, kind="ExternalInput").ap(),
        "fix": nc.dram_tensor("fix", [nkeys, 1], F32, kind="ExternalInput").ap(),
        "Wq": nc.dram_tensor("Wq", [MID, QDIM], F32, kind="ExternalInput").ap(),
        "bq": nc.dram_tensor("bq", [MID, 1], F32, kind="ExternalInput").ap(),
        "Wk": nc.dram_tensor("Wk", [MID, QDIM], F32, kind="ExternalInput").ap(),
        "bk": nc.dram_tensor("bk", [MID, 1], F32, kind="ExternalInput").ap(),
        "out": nc.dram_tensor("out", [nq, QDIM], F32, kind="ExternalOutput").ap(),
    }


def emit_kernel(tc, nq, nkeys, q_group=512, mm_dt=F32R, mask_mode="fp8",
                lookahead=2, io=None):
    """Emit the per-core program. nq = queries this core, nkeys = all keys."""
    nc = tc.nc
    n_qt = nq // 128          # query 128-tiles
    n_kt = nkeys // 128       # key 128-tiles
    qg = min(q_group, nq)     # q columns per PSUM slab
    n_qg = nq // qg
    n_qc = qg // 128          # 128-chunks per q group
    inv_sqrt_mid = 1.0 / math.sqrt(MID)
    vw = QDIM + 2             # V' width: 256 dims + ones col + pad (even for f32r)

    if io is None:
        io = declare_io(nc, nq, nkeys)
    main, maskd, other, fix = io["main"], io["mask"], io["other"], io["fix"]
    wq, bq, wk, bk, out = io["Wq"], io["bq"], io["Wk"], io["bk"], io["out"]

    # mask viewed as [qg-group, kt-group, partition(q), qc-chunk, k]
    KTG = min(16, n_kt)    # key tiles per mask DMA (2KB contiguous chunks)
    mask_rg = maskd.rearrange(
        "(qh qc p) (ktg k) -> qh ktg p qc k", qc=n_qc, p=128, k=KTG * 128
    )
    n_qc2 = max(1, qg // 256)  # 256-row chunks for DoubleRow mask MMs
    mask_dr = maskd.rearrange(
        "(qh qc2 p j) (ktg k) -> qh ktg p qc2 j k",
        qc2=n_qc2, p=qg // (2 * n_qc2), j=2, k=KTG * 128,
    )

    ident = mybir.ActivationFunctionType.Identity
    expf = mybir.ActivationFunctionType.Exp

    with (
        tc.tile_pool(name="const", bufs=1) as constp,
        tc.tile_pool(name="big", bufs=1) as bigp,
        tc.tile_pool(name="mwork", bufs=1) as mwork,
    ):
        # ---- constants (ident/diag first: no DMA dependency) ----
        ident_f32 = constp.tile([128, 128], F32)
        _diag(nc, ident_f32, 1.0)
        if mm_dt == F32:
            ident_t = ident_f32
        else:
            ident_t = constp.tile([128, 128], mm_dt)
            nc.vector.tensor_copy(ident_t, ident_f32)
        if mask_mode == "fp8":
            diag_mm = constp.tile([128, 128], BF16)
            _diag(nc, diag_mm, NEG_BIG * F8_SUB)  # f8 byte 0x01 -> 2^-9
        else:  # fp8dr
            diag_dr = constp.tile([128, 2, 256], F8E4)
            nc.gpsimd.memset(diag_dr, 0.0)
            # fill where 2*ki + j - q' == 0
            nc.gpsimd.affine_select(
                out=diag_dr, in_=diag_dr,
                compare_op=mybir.AluOpType.not_equal,
                fill=-F8_MAX, base=0,
                pattern=[[1, 2], [-1, 256]],
                channel_multiplier=2,
            )

        wq_s = constp.tile([MID, QDIM], mm_dt)
        wk_s = constp.tile([MID, QDIM], mm_dt)
        bq_s = constp.tile([MID, 1], F32)
        nc.sync.dma_start(bq_s, bq)
        if mask_mode == "fp8dr":
            bq_eps = constp.tile([MID, 1], F32)
            nc.vector.tensor_scalar_mul(bq_eps, bq_s, EPS_DR)
            q_bias, q_scale = bq_eps, EPS_DR
            exp_scale = inv_sqrt_mid / EPS_DR
        else:
            q_bias, q_scale = bq_s, 1.0
            exp_scale = inv_sqrt_mid
        bk_s = constp.tile([MID, 1], F32)
        nc.sync.dma_start(bk_s, bk)

        # ---- persistent big tensors ----
        kt_sb = bigp.tile([MID, nkeys], mm_dt)      # K^T, dim-major
        qt_sb = bigp.tile([MID, nq], mm_dt)         # Q^T, dim-major
        vp_sb = bigp.tile([128, n_kt, vw], mm_dt)   # V' tiles, token-major
        nc.scalar.activation(vp_sb[:, :, QDIM : QDIM + 2], vp_sb[:, :, 0:2],
                             mybir.ActivationFunctionType.Copy,
                             bias=1.0, scale=0.0)

        # ---- mask prefetch machinery (pool stays open for the main loop) ----
        n_ktg = n_kt // KTG
        mask_bufs = 3
        mask_tiles = {}

        def fetch_mask(qh, ktg):
            if mask_mode == "fp8dr":
                mg = mwork.tile([128, n_qc2, 2, KTG * 128], F8E4,
                                name="mask_gdr", tag="mg", bufs=mask_bufs)
                for qc2 in range(n_qc2):
                    nc.sync.dma_start(
                        mg[:, qc2], mask_dr[qh, ktg][:, qc2].bitcast(F8E4))
            else:
                mg = mwork.tile([128, n_qc, KTG * 128], F8E4,
                                name="mask_gf8", tag="mg", bufs=mask_bufs)
                nc.sync.dma_start(mg, mask_rg[qh, ktg].bitcast(F8E4))
            mask_tiles[(qh, ktg)] = mg

        wqt_s = constp.tile([128, 2, MID], mm_dt)
        wkt_s = constp.tile([128, 2, MID], mm_dt)
        fix_s = constp.tile([128, n_kt], F32)
        out_r = out.rearrange("(qh qc p) d -> qh p qc d", qc=n_qc, p=128)

        with (
            tc.tile_pool(name="prologue", bufs=4) as prop,
            tc.tile_pool(name="pps", bufs=1, space="PSUM") as pps,
            tc.tile_pool(name="avpsum", bufs=1, space="PSUM") as avpsum,
            tc.tile_pool(name="ework", bufs=4) as ework,
            tc.tile_pool(name="outp", bufs=2) as outp,
        ):
            # One shared 3-deep rotation of 2KB PSUM banks serves the
            # transpose staging tiles AND the attention slabs, so K-prep can
            # interleave with the qh=0 attention loop inside 8 PSUM banks
            # (3 shared + 1 proj + 4 AV accumulators).
            def ps_tile(shape, dtype, name):
                return pps.tile(shape, dtype, name=name, tag="ps", bufs=3)

            def mps_tile(shape, name):
                return pps.tile(shape, F32, name=name, tag="mps", bufs=1)

            warm_ps = ps_tile([128, 4, 128], mm_dt, "warm_ps")
            for w in range(4):
                nc.tensor.transpose(warm_ps[:, w, :], ident_t, ident_t)

            # ---- Q^T = Wq @ main^T + bq  (pairs of 128-tiles: 256 moving) ----
            # main tiles DMA'd ahead of the weights so the PE's first
            # transposes start as early as possible
            main_ts = []
            for t2 in range(n_qt // 2):
                main_t = prop.tile([128, 2, QDIM], mm_dt, name="main_t", tag="in")
                nc.sync.dma_start(
                    main_t, main[t2 * 256 : (t2 + 1) * 256, :]
                    .rearrange("(a p) d -> p a d", p=128).bitcast(mm_dt)
                )
                main_ts.append(main_t)
                if t2 == 0:
                    nc.sync.dma_start(wq_s, wq.bitcast(mm_dt))
                    nc.sync.dma_start(wk_s, wk.bitcast(mm_dt))
            for t2 in range(n_qt // 2):
                main_t = main_ts[t2]
                tp_ps = ps_tile([128, 4, 128], mm_dt, "tp_ps")
                for h in range(2):
                    for a in range(2):
                        nc.tensor.transpose(
                            tp_ps[:, 2 * h + a, :],
                            main_t[:, a, h * 128 : (h + 1) * 128], ident_t,
                        )
                maint_s = prop.tile([128, 4, 128], mm_dt, name="maint_s", tag="tr")
                nc.vector.tensor_copy(maint_s, tp_ps)
                if t2 == 0:
                    # WqT / WkT transposes slot in behind the first Q tile
                    wt_ps = ps_tile([128, 4, MID], mm_dt, "wt_ps")
                    for h in range(2):
                        nc.tensor.transpose(
                            wt_ps[:, h, :],
                            wq_s[:, h * 128 : (h + 1) * 128], ident_t)
                        nc.tensor.transpose(
                            wt_ps[:, 2 + h, :],
                            wk_s[:, h * 128 : (h + 1) * 128], ident_t)
                    nc.vector.tensor_copy(wqt_s, wt_ps[:, 0:2, :])
                    nc.vector.tensor_copy(wkt_s, wt_ps[:, 2:4, :])
                q_ps = mps_tile([MID, 256], "q_ps")
                for h in range(2):
                    nc.tensor.matmul(
                        q_ps,
                        wqt_s[:, h, :],
                        maint_s[:, 2 * h : 2 * h + 2, :],
                        start=(h == 0),
                        stop=(h == 1),
                    )
                nc.scalar.activation(
                    qt_sb[:, t2 * 256 : (t2 + 1) * 256], q_ps, ident,
                    bias=q_bias, scale=q_scale,
                )

            # fix loaded with one contiguous DMA [tile, 128] then PE-transposed
            # to the per-partition layout fix_s[p, kt] = fix[kt*128 + p]
            assert n_kt <= 128
            fix_tT = prop.tile([n_kt, 128], F32, name="fix_tT", tag="fT")
            nc.sync.dma_start(
                fix_tT, fix.rearrange("(t p) d -> t (p d)", p=128)
            )
            ft_ps = mps_tile([128, n_kt], "ft_ps")
            nc.tensor.transpose(ft_ps, fix_tT, ident_f32[0:n_kt, 0:n_kt])
            nc.vector.tensor_copy(fix_s, ft_ps)

            fetch_mask(0, 0)

            # ---- K-prep: K^T = Wk @ other^T + bk ; V' = [fix*other | 1] ----
            # other rows are DMA'd straight into vp_sb; the PE transposes
            # read the raw rows from there, after which gpsimd scales them
            # by fix in place (ordered by the tile dep tracker). Split in two
            # halves so the projection matmul (which needs the DVE-evacuated
            # transposes) is emitted an attention-stage later than the
            # transposes themselves.
            kprep_st = {}

            def kprep_t(k2):
                other_t = vp_sb[:, 2 * k2 : 2 * k2 + 2, 0:QDIM]
                nc.sync.dma_start(
                    other_t, other[k2 * 256 : (k2 + 1) * 256, :]
                    .rearrange("(a p) d -> p a d", p=128).bitcast(mm_dt)
                )
                to_ps = ps_tile([128, 4, 128], mm_dt, "to_ps")
                for h in range(2):
                    for a in range(2):
                        nc.tensor.transpose(
                            to_ps[:, 2 * h + a, :],
                            other_t[:, a, h * 128 : (h + 1) * 128], ident_t,
                        )
                for a in range(2):
                    kt = 2 * k2 + a
                    nc.gpsimd.tensor_scalar_mul(
                        vp_sb[:, kt, 0:QDIM],
                        other_t[:, a, :],
                        fix_s[:, kt : kt + 1],
                    )
                ot_s = prop.tile([128, 4, 128], mm_dt, name="ot_s", tag="tr")
                nc.vector.tensor_copy(ot_s, to_ps)
                kprep_st[k2] = ot_s

            def kprep_p(k2):
                ot_s = kprep_st.pop(k2)
                k_ps = mps_tile([MID, 256], "k_ps")
                for h in range(2):
                    nc.tensor.matmul(
                        k_ps,
                        wkt_s[:, h, :],
                        ot_s[:, 2 * h : 2 * h + 2, :],
                        start=(h == 0),
                        stop=(h == 1),
                    )
                nc.scalar.activation(
                    kt_sb[:, k2 * 256 : (k2 + 1) * 256], k_ps, ident, bias=bk_s
                )

            def stage_a(qh, kt):
                """Mask + QK matmuls into a fresh PSUM slab, then exp."""
                ktg, kti = divmod(kt, KTG)
                if (qh, ktg) not in mask_tiles:
                    fetch_mask(qh, ktg)
                if kti == 0:
                    nqh, ngt = (qh, ktg + 1) if ktg + 1 < n_ktg else (qh + 1, 0)
                    if nqh < n_qg and (nqh, ngt) not in mask_tiles:
                        fetch_mask(nqh, ngt)
                mg = mask_tiles[(qh, ktg)]
                attn_ps = ps_tile([128, qg], F32, "attn_ps")
                if mask_mode == "fp8dr":
                    for qc2 in range(n_qc2):
                        nc.tensor.matmul(
                            attn_ps[:, qc2 * 256 : (qc2 + 1) * 256],
                            mg[:, qc2, :, kti * 128 : (kti + 1) * 128],
                            diag_dr,
                            start=(qc2 == 0),
                            stop=False,
                            perf_mode=mybir.MatmulPerfMode.DoubleRow,
                            skip_group_check=True,
                        )
                else:
                    for qc in range(n_qc):
                        nc.tensor.matmul(
                            attn_ps[:, qc * 128 : (qc + 1) * 128],
                            mg[:, qc, kti * 128 : (kti + 1) * 128],
                            diag_mm,
                            start=(qc == 0),
                            stop=False,
                            skip_group_check=True,
                        )
                nc.tensor.matmul(
                    attn_ps,
                    kt_sb[:, kt * 128 : (kt + 1) * 128],
                    qt_sb[:, qh * qg : (qh + 1) * qg],
                    start=False,
                    stop=True,
                    skip_group_check=True,
                )
                expattn = ework.tile([128, qg], BF16, name="expattn")
                nc.scalar.activation(expattn, attn_ps, expf, scale=exp_scale)
                return expattn

            def stage_av(av_ps, kt, expattn):
                for qc in range(n_qc):
                    nc.tensor.matmul(
                        av_ps[qc],
                        expattn[:, qc * 128 : (qc + 1) * 128],
                        vp_sb[:, kt, :],
                        start=(kt == 0),
                        stop=(kt == n_kt - 1),
                    )

            def out_stage(qh, av_ps):
                out_t = outp.tile([128, n_qc, QDIM], F32, name="out_t", tag="ot")
                h = n_qc // 2
                for qc in range(n_qc):
                    recip = outp.tile([128, 1], F32, name="recip", tag="rc")
                    nc.vector.reciprocal_approx_fast(
                        recip, av_ps[qc][:, QDIM : QDIM + 1])
                    if qc % 2 == 0:
                        nc.vector.tensor_scalar_mul(
                            out_t[:, qc, :], av_ps[qc][:, 0:QDIM], recip)
                    else:
                        nc.scalar.activation(
                            out_t[:, qc, :], av_ps[qc][:, 0:QDIM], ident,
                            scale=recip)
                    if qc == h - 1:
                        nc.sync.dma_start(out_r[qh][:, 0:h, :],
                                          out_t[:, 0:h, :])
                nc.sync.dma_start(out_r[qh][:, h:n_qc, :], out_t[:, h:n_qc, :])

            # ---- qh = 0: attention fused with K-prep (one pair ahead) ----
            n_k2 = n_kt // 2
            kprep_t(0)
            kprep_p(0)
            av_ps = [
                avpsum.tile([128, vw], F32, tag=f"av{qc}", name=f"av{qc}")
                for qc in range(n_qc)
            ]
            pend = []
            for kt in range(n_kt):
                k2, r = divmod(kt, 2)
                if r == 0 and k2 + 1 < n_k2:
                    kprep_t(k2 + 1)
                pend.append((kt, stage_a(0, kt)))
                if r == 0 and k2 + 1 < n_k2:
                    kprep_p(k2 + 1)
                if len(pend) > 1:
                    stage_av(av_ps, *pend.pop(0))
            for item in pend:
                stage_av(av_ps, *item)
            out_stage(0, av_ps)

            # ---- remaining q groups: plain pipelined attention ----
            for qh in range(1, n_qg):
                av_ps = [
                    avpsum.tile([128, vw], F32, tag=f"av{qc}", name=f"av{qc}")
                    for qc in range(n_qc)
                ]
                pend = []
                for kt in range(n_kt):
                    pend.append((kt, stage_a(qh, kt)))
                    if len(pend) > lookahead:
                        stage_av(av_ps, *pend.pop(0))
                for item in pend:
                    stage_av(av_ps, *item)
                out_stage(qh, av_ps)


def build_nc(nq, nkeys, q_group=512, mm_dt=F32R, mask_mode=None, repeat=1,
             lookahead=2):
    if mask_mode is None:
        mask_mode = os.environ.get("ADJ_MASK_MODE", "fp8")
    nc = bacc.Bacc("TRN2", target_bir_lowering=False, debug=False,
                   enable_asserts=False)
    io = declare_io(nc, nq, nkeys)
    with tile.TileContext(nc) as tc:
        for _ in range(repeat):
            emit_kernel(tc, nq, nkeys, q_group=q_group, mm_dt=mm_dt,
                        mask_mode=mask_mode, lookahead=lookahead, io=io)
    nc.compile()
    return nc


def make_in_maps(inputs, n_cores=N_CORES):
    """Shard full inputs into per-core input maps."""
    main_feat = np.ascontiguousarray(np.asarray(inputs["main_feat"], dtype=np.float32))
    other_feat = np.ascontiguousarray(np.asarray(inputs["other_feat"], dtype=np.float32))
    fix_feat = np.ascontiguousarray(
        np.asarray(inputs["fix_feat"], dtype=np.float32).reshape(-1, 1)
    )
    mask = np.ascontiguousarray(np.asarray(inputs["mask"])).view(np.uint8)
    wq_ = np.ascontiguousarray(np.asarray(inputs["Wq"], dtype=np.float32))
    bq_ = np.ascontiguousarray(np.asarray(inputs["bq"], dtype=np.float32).reshape(-1, 1))
    wk_ = np.ascontiguousarray(np.asarray(inputs["Wk"], dtype=np.float32))
    bk_ = np.ascontiguousarray(np.asarray(inputs["bk"], dtype=np.float32).reshape(-1, 1))

    n = main_feat.shape[0]
    per = n // n_cores
    in_maps = []
    for c in range(n_cores):
        sl = slice(c * per, (c + 1) * per)
        in_maps.append(
            {
                "main": np.ascontiguousarray(main_feat[sl]),
                "mask": np.ascontiguousarray(mask[sl]),
                "other": other_feat,
                "fix": fix_feat,
                "Wq": wq_,
                "bq": bq_,
                "Wk": wk_,
                "bk": bk_,
            }
        )
    return in_maps


_NC_CACHE = {}


def _get_nc(nq, nkeys):
    key = (nq, nkeys)
    if key not in _NC_CACHE:
        _NC_CACHE[key] = build_nc(nq, nkeys)
    return _NC_CACHE[key]


class _Executor:
    """Cached jit(shard_map) wrapper around the compiled Bass module so
    repeated kernel() calls skip retracing/recompiling."""

    def __init__(self, nc, n_cores=N_CORES):
        import jax
        from jax.sharding import Mesh, PartitionSpec
        from jax.experimental.shard_map import shard_map
        from concourse import bass2jax
        from concourse.bass2jax import _bass_exec_p, install_neuronx_cc_hook

        install_neuronx_cc_hook()
        self.n_cores = n_cores
        partition_name = (
            nc.partition_id_tensor.name if nc.partition_id_tensor else None
        )
        in_names, out_names, out_avals = [], [], []
        for alloc in nc.m.functions[0].allocations:
            if not isinstance(alloc, mybir.MemoryLocationSet):
                continue
            name = alloc.memorylocations[0].name
            if alloc.kind == "ExternalInput":
                if name != partition_name:
                    in_names.append(name)
            elif alloc.kind == "ExternalOutput":
                out_names.append(name)
                out_avals.append(
                    jax.core.ShapedArray(
                        tuple(alloc.tensor_shape), mybir.dt.np(alloc.dtype)
                    )
                )
        self.in_names = list(in_names)
        self.out_names = out_names
        self.out_avals = out_avals
        all_names = in_names + out_names
        if partition_name is not None:
            all_names.append(partition_name)

        def _body(*args):
            operands = list(args)
            if partition_name is not None:
                operands.append(bass2jax.partition_id_tensor())
            return tuple(
                _bass_exec_p.bind(
                    *operands,
                    out_avals=tuple(out_avals),
                    in_names=tuple(all_names),
                    out_names=tuple(out_names),
                    lowering_input_output_aliases=(),
                    sim_require_finite=True,
                    sim_require_nnan=True,
                    nc=nc,
                )
            )

        devices = jax.devices()[:n_cores]
        self.mesh = Mesh(np.asarray(devices), ("core",))
        n_args = len(self.in_names) + len(out_names)
        self.f = jax.jit(
            shard_map(
                _body,
                mesh=self.mesh,
                in_specs=(PartitionSpec("core"),) * n_args,
                out_specs=(PartitionSpec("core"),) * len(out_names),
                check_rep=False,
            ),
            keep_unused=True,
        )

    def run(self, in_maps):
        concat_in = [
            np.concatenate([m[nm] for m in in_maps], axis=0)
            for nm in self.in_names
        ]
        concat_zeros = [
            np.zeros((self.n_cores * a.shape[0], *a.shape[1:]), a.dtype)
            for a in self.out_avals
        ]
        r = self.f(*concat_in, *concat_zeros)
        return np.asarray(r[0])


_EXEC_CACHE = {}


def _get_executor(nq, nkeys):
    key = (nq, nkeys)
    if key not in _EXEC_CACHE:
        _EXEC_CACHE[key] = _Executor(_get_nc(nq, nkeys))
    return _EXEC_CACHE[key]


def kernel(**inputs) -> np.ndarray:
    n = np.asarray(inputs["main_feat"]).shape[0]
    nkeys = np.asarray(inputs["other_feat"]).shape[0]
    in_maps = make_in_maps(inputs, N_CORES)
    try:
        ex = _get_executor(n // N_CORES, nkeys)
        return ex.run(in_maps)
    except Exception:
        nc = _get_nc(n // N_CORES, nkeys)
        res = run_bass_kernel_spmd(nc, in_maps, core_ids=list(range(N_CORES)))
        return np.concatenate(
            [res.results[c]["out"] for c in range(N_CORES)], axis=0
        )


# revision 18
# speedup vs baseline: 1.4571x; 1.1141x over previous
"""Trainium2 Bass kernel for AdjAttenAgger-style masked cross-attention.

Computes, for full inputs:
    Q = main_feat @ Wq.T + bq              # [N, MID]
    K = other_feat @ Wk.T + bk             # [M, MID]
    attn = softmax(where(mask, -BIG, Q K^T / sqrt(MID)), axis=-1)
    out  = attn @ (fix_feat[:, None] * other_feat)          # [N, KDIM]

Sharding: rows of main_feat/mask (the N query axis) are split across 8
NeuronCores; other_feat/fix_feat/weights are replicated. No collectives.

Per-core dataflow (all layouts chosen so no large tensor is ever
transposed outside the PE array):
  - QT [MID, nq] and KT [MID, nk] are built dim-major via PE-transposed
    input tiles, so the QK^T matmul directly produces attnT [k, q] slabs.
  - The boolean mask (q-major in DRAM, only efficiently loadable q-major)
    is applied *by the PE*: accumulating matmuls with the q-major mask
    tile as the stationary operand and a scaled diagonal as the moving
    operand add -BIG * mask^T into the attnT PSUM tile.
  - ACT computes exp((attnT - BIG*mask)/sqrt(MID)) PSUM->SBUF; no row-max
    subtraction is needed (logits are O(1); masked entries underflow to 0).
  - V' = [fix*other | 1] has an extra ones column, so the attn@V' matmul
    also produces the softmax denominators; a per-row divide finishes the
    softmax normalization on the [nq, 256] output only.

Scheduling: the prologue batches the four PE transposes of each 256-row
input chunk into a single PSUM bank evacuated by one DVE copy, and the
main loop emits the mask/QK matmuls of k-tile kt+2 before the AV matmuls
of k-tile kt, so the PE never stalls on the exp activation.
"""

import math
import os

import numpy as np

import concourse.bass as bass
from concourse import bacc
import concourse.mybir as mybir
import concourse.tile as tile
from concourse.bass_utils import run_bass_kernel_spmd

F32 = mybir.dt.float32
F32R = mybir.dt.float32r
BF16 = mybir.dt.bfloat16
U8 = mybir.dt.uint8
F8E4 = mybir.dt.float8e4

N_CORES = 8
QDIM = 256       # main/other feature dim
MID = 128
NEG_BIG = -float(2 ** 41)  # additive pre-scale mask value; exp() underflows to 0
F8_SUB = 2.0 ** -9         # value of byte 0x01 reinterpreted as float8e4 (e4m3)
F8_MAX = 240.0             # fp8e4 (IEEE e4m3) max normal
EPS_DR = 2.0 ** -12        # Q prescale for fp8x8 DoubleRow mask (product -0.875)


def _diag(nc, ap, fill):
    """ap[i, j] = fill if i == j else 0."""
    nc.gpsimd.memset(ap, 0.0)
    nc.gpsimd.affine_select(
        out=ap, in_=ap,
        compare_op=mybir.AluOpType.not_equal,
        fill=fill, base=0,
        pattern=[[-1, ap.shape[1]]],
        channel_multiplier=1,
    )


def declare_io(nc, nq, nkeys):
    return {
        "main": nc.dram_tensor("main", [nq, QDIM], F32, kind="ExternalInput").ap(),
        "mask": nc.dram_tensor("mask", [nq, nkeys], U8, kind="ExternalInput").ap(),
        "other": nc.dram_tensor("other", [nkeys, QDIM], F32, kind="ExternalInput").ap(),
        "fix": nc.dram_tensor("fix", [nkeys, 1], F32, kind="ExternalInput").ap(),
        "Wq": nc.dram_tensor("Wq", [MID, QDIM], F32, kind="ExternalInput").ap(),
        "bq": nc.dram_tensor("bq", [MID, 1], F32, kind="ExternalInput").ap(),
        "Wk": nc.dram_tensor("Wk", [MID, QDIM], F32, kind="ExternalInput").ap(),
        "bk": nc.dram_tensor("bk", [MID, 1], F32, kind="ExternalInput").ap(),
        "out": nc.dram_tensor("out", [nq, QDIM], F32, kind="ExternalOutput").ap(),
    }


def emit_kernel(tc, nq, nkeys, q_group=512, mm_dt=F32R, mask_mode="fp8",
                lookahead=2, io=None):
    """Emit the per-core program. nq = queries this core, nkeys = all keys."""
    nc = tc.nc
    n_qt = nq // 128          # query 128-tiles
    n_kt = nkeys // 128       # key 128-tiles
    qg = min(q_group, nq)     # q columns per PSUM slab
    n_qg = nq // qg
    n_qc = qg // 128          # 128-chunks per q group
    inv_sqrt_mid = 1.0 / math.sqrt(MID)
    vw = QDIM + 2             # V' width: 256 dims + ones col + pad (even for f32r)

    if io is None:
        io = declare_io(nc, nq, nkeys)
    main, maskd, other, fix = io["main"], io["mask"], io["other"], io["fix"]
    wq, bq, wk, bk, out = io["Wq"], io["bq"], io["Wk"], io["bk"], io["out"]

    # mask viewed as [qg-group, kt-group, partition(q), qc-chunk, k]
    KTG = min(16, n_kt)    # key tiles per mask DMA (2KB contiguous chunks)
    mask_rg = maskd.rearrange(
        "(qh qc p) (ktg k) -> qh ktg p qc k", qc=n_qc, p=128, k=KTG * 128
    )
    n_qc2 = max(1, qg // 256)  # 256-row chunks for DoubleRow mask MMs
    mask_dr = maskd.rearrange(
        "(qh qc2 p j) (ktg k) -> qh ktg p qc2 j k",
        qc2=n_qc2, p=qg // (2 * n_qc2), j=2, k=KTG * 128,
    )

    ident = mybir.ActivationFunctionType.Identity
    expf = mybir.ActivationFunctionType.Exp

    with (
        tc.tile_pool(name="const", bufs=1) as constp,
        tc.tile_pool(name="big", bufs=1) as bigp,
        tc.tile_pool(name="mwork", bufs=1) as mwork,
    ):
        # ---- constants (ident/diag first: no DMA dependency) ----
        ident_f32 = constp.tile([128, 128], F32)
        _diag(nc, ident_f32, 1.0)
        if mm_dt == F32:
            ident_t = ident_f32
        else:
            ident_t = constp.tile([128, 128], mm_dt)
            nc.vector.tensor_copy(ident_t, ident_f32)
        if mask_mode == "fp8":
            diag_mm = constp.tile([128, 128], BF16)
            _diag(nc, diag_mm, NEG_BIG * F8_SUB)  # f8 byte 0x01 -> 2^-9
        else:  # fp8dr
            diag_dr = constp.tile([128, 2, 256], F8E4)
            nc.gpsimd.memset(diag_dr, 0.0)
            # fill where 2*ki + j - q' == 0
            nc.gpsimd.affine_select(
                out=diag_dr, in_=diag_dr,
                compare_op=mybir.AluOpType.not_equal,
                fill=-F8_MAX, base=0,
                pattern=[[1, 2], [-1, 256]],
                channel_multiplier=2,
            )

        wq_s = constp.tile([MID, QDIM], mm_dt)
        wk_s = constp.tile([MID, QDIM], mm_dt)
        bq_s = constp.tile([MID, 1], F32)
        nc.sync.dma_start(bq_s, bq)
        if mask_mode == "fp8dr":
            bq_eps = constp.tile([MID, 1], F32)
            nc.vector.tensor_scalar_mul(bq_eps, bq_s, EPS_DR)
            q_bias, q_scale = bq_eps, EPS_DR
            exp_scale = inv_sqrt_mid / EPS_DR
        else:
            q_bias, q_scale = bq_s, 1.0
            exp_scale = inv_sqrt_mid
        bk_s = constp.tile([MID, 1], F32)
        nc.sync.dma_start(bk_s, bk)

        # ---- persistent big tensors ----
        kt_sb = bigp.tile([MID, nkeys], mm_dt)      # K^T, dim-major
        qt_sb = bigp.tile([MID, nq], mm_dt)         # Q^T, dim-major
        vp_sb = bigp.tile([128, n_kt, vw], mm_dt)   # V' tiles, token-major
        nc.scalar.activation(vp_sb[:, :, QDIM : QDIM + 2], vp_sb[:, :, 0:2],
                             mybir.ActivationFunctionType.Copy,
                             bias=1.0, scale=0.0)

        # ---- mask prefetch machinery (pool stays open for the main loop) ----
        n_ktg = n_kt // KTG
        mask_bufs = 3
        mask_tiles = {}

        def fetch_mask(qh, ktg):
            if mask_mode == "fp8dr":
                mg = mwork.tile([128, n_qc2, 2, KTG * 128], F8E4,
                                name="mask_gdr", tag="mg", bufs=mask_bufs)
                for qc2 in range(n_qc2):
                    nc.sync.dma_start(
                        mg[:, qc2], mask_dr[qh, ktg][:, qc2].bitcast(F8E4))
            else:
                mg = mwork.tile([128, n_qc, KTG * 128], F8E4,
                                name="mask_gf8", tag="mg", bufs=mask_bufs)
                nc.sync.dma_start(mg, mask_rg[qh, ktg].bitcast(F8E4))
            mask_tiles[(qh, ktg)] = mg

        wqt_s = constp.tile([128, 2, MID], mm_dt)
        wkt_s = constp.tile([128, 2, MID], mm_dt)
        fix_s = constp.tile([128, n_kt], F32)
        out_r = out.rearrange("(qh qc p) d -> qh p qc d", qc=n_qc, p=128)

        with (
            tc.tile_pool(name="prologue", bufs=4) as prop,
            tc.tile_pool(name="pps", bufs=1, space="PSUM") as pps,
            tc.tile_pool(name="avpsum", bufs=1, space="PSUM") as avpsum,
            tc.tile_pool(name="ework", bufs=4) as ework,
            tc.tile_pool(name="outp", bufs=2) as outp,
        ):
            # One shared 3-deep rotation of 2KB PSUM banks serves the
            # transpose staging tiles AND the attention slabs, so K-prep can
            # interleave with the qh=0 attention loop inside 8 PSUM banks
            # (3 shared + 1 proj + 4 AV accumulators).
            def ps_tile(shape, dtype, name):
                return pps.tile(shape, dtype, name=name, tag="ps", bufs=3)

            def mps_tile(shape, name):
                return pps.tile(shape, F32, name=name, tag="mps", bufs=1)

            warm_ps = ps_tile([128, 4, 128], mm_dt, "warm_ps")
            for w in range(4):
                nc.tensor.transpose(warm_ps[:, w, :], ident_t, ident_t)

            # ---- Q^T = Wq @ main^T + bq  (pairs of 128-tiles: 256 moving) ----
            # main tiles DMA'd ahead of the weights so the PE's first
            # transposes start as early as possible
            main_ts = []
            for t2 in range(n_qt // 2):
                main_t = prop.tile([128, 2, QDIM], mm_dt, name="main_t", tag="in")
                nc.sync.dma_start(
                    main_t, main[t2 * 256 : (t2 + 1) * 256, :]
                    .rearrange("(a p) d -> p a d", p=128).bitcast(mm_dt)
                )
                main_ts.append(main_t)
                if t2 == 0:
                    nc.sync.dma_start(wq_s, wq.bitcast(mm_dt))
                    nc.sync.dma_start(wk_s, wk.bitcast(mm_dt))
            for t2 in range(n_qt // 2):
                main_t = main_ts[t2]
                tp_ps = ps_tile([128, 4, 128], mm_dt, "tp_ps")
                for h in range(2):
                    for a in range(2):
                        nc.tensor.transpose(
                            tp_ps[:, 2 * h + a, :],
                            main_t[:, a, h * 128 : (h + 1) * 128], ident_t,
                        )
                maint_s = prop.tile([128, 4, 128], mm_dt, name="maint_s", tag="tr")
                nc.vector.tensor_copy(maint_s, tp_ps)
                if t2 == 0:
                    # WqT / WkT transposes slot in behind the first Q tile
                    wt_ps = ps_tile([128, 4, MID], mm_dt, "wt_ps")
                    for h in range(2):
                        nc.tensor.transpose(
                            wt_ps[:, h, :],
                            wq_s[:, h * 128 : (h + 1) * 128], ident_t)
                        nc.tensor.transpose(
                            wt_ps[:, 2 + h, :],
                            wk_s[:, h * 128 : (h + 1) * 128], ident_t)
                    nc.vector.tensor_copy(wqt_s, wt_ps[:, 0:2, :])
                    nc.vector.tensor_copy(wkt_s, wt_ps[:, 2:4, :])
                q_ps = mps_tile([MID, 256], "q_ps")
                for h in range(2):
                    nc.tensor.matmul(
                        q_ps,
                        wqt_s[:, h, :],
                        maint_s[:, 2 * h : 2 * h + 2, :],
                        start=(h == 0),
                        stop=(h == 1),
                    )
                nc.scalar.activation(
                    qt_sb[:, t2 * 256 : (t2 + 1) * 256], q_ps, ident,
                    bias=q_bias, scale=q_scale,
                )

            # fix loaded with one contiguous DMA [tile, 128] then PE-transposed
            # to the per-partition layout fix_s[p, kt] = fix[kt*128 + p]
            assert n_kt <= 128
            fix_tT = prop.tile([n_kt, 128], F32, name="fix_tT", tag="fT")
            nc.sync.dma_start(
                fix_tT, fix.rearrange("(t p) d -> t (p d)", p=128)
            )
            ft_ps = mps_tile([128, n_kt], "ft_ps")
            nc.tensor.transpose(ft_ps, fix_tT, ident_f32[0:n_kt, 0:n_kt])
            nc.vector.tensor_copy(fix_s, ft_ps)

            fetch_mask(0, 0)

            # ---- K-prep: K^T = Wk @ other^T + bk ; V' = [fix*other | 1] ----
            # other rows are DMA'd straight into vp_sb; the PE transposes
            # read the raw rows from there, after which gpsimd scales them
            # by fix in place (ordered by the tile dep tracker). Split in two
            # halves so the projection matmul (which needs the DVE-evacuated
            # transposes) is emitted an attention-stage later than the
            # transposes themselves.
            kprep_st = {}

            def kprep_t(k2):
                other_t = vp_sb[:, 2 * k2 : 2 * k2 + 2, 0:QDIM]
                nc.sync.dma_start(
                    other_t, other[k2 * 256 : (k2 + 1) * 256, :]
                    .rearrange("(a p) d -> p a d", p=128).bitcast(mm_dt)
                )
                to_ps = ps_tile([128, 4, 128], mm_dt, "to_ps")
                for h in range(2):
                    for a in range(2):
                        nc.tensor.transpose(
                            to_ps[:, 2 * h + a, :],
                            other_t[:, a, h * 128 : (h + 1) * 128], ident_t,
                        )
                for a in range(2):
                    kt = 2 * k2 + a
                    nc.gpsimd.tensor_scalar_mul(
                        vp_sb[:, kt, 0:QDIM],
                        other_t[:, a, :],
                        fix_s[:, kt : kt + 1],
                    )
                ot_s = prop.tile([128, 4, 128], mm_dt, name="ot_s", tag="tr")
                nc.vector.tensor_copy(ot_s, to_ps)
                kprep_st[k2] = ot_s

            def kprep_p(k2):
                ot_s = kprep_st.pop(k2)
                k_ps = mps_tile([MID, 256], "k_ps")
                for h in range(2):
                    nc.tensor.matmul(
                        k_ps,
                        wkt_s[:, h, :],
                        ot_s[:, 2 * h : 2 * h + 2, :],
                        start=(h == 0),
                        stop=(h == 1),
                    )
                nc.scalar.activation(
                    kt_sb[:, k2 * 256 : (k2 + 1) * 256], k_ps, ident, bias=bk_s
                )

            def stage_a(qh, kt):
                """Mask + QK matmuls into a fresh PSUM slab, then exp."""
                ktg, kti = divmod(kt, KTG)
                if (qh, ktg) not in mask_tiles:
                    fetch_mask(qh, ktg)
                if kti == 0:
                    nqh, ngt = (qh, ktg + 1) if ktg + 1 < n_ktg else (qh + 1, 0)
                    if nqh < n_qg and (nqh, ngt) not in mask_tiles:
                        fetch_mask(nqh, ngt)
                mg = mask_tiles[(qh, ktg)]
                attn_ps = ps_tile([128, qg], F32, "attn_ps")
                if mask_mode == "fp8dr":
                    for qc2 in range(n_qc2):
                        nc.tensor.matmul(
                            attn_ps[:, qc2 * 256 : (qc2 + 1) * 256],
                            mg[:, qc2, :, kti * 128 : (kti + 1) * 128],
                            diag_dr,
                            start=(qc2 == 0),
                            stop=False,
                            perf_mode=mybir.MatmulPerfMode.DoubleRow,
                            skip_group_check=True,
                        )
                else:
                    for qc in range(n_qc):
                        nc.tensor.matmul(
                            attn_ps[:, qc * 128 : (qc + 1) * 128],
                            mg[:, qc, kti * 128 : (kti + 1) * 128],
                            diag_mm,
                            start=(qc == 0),
                            stop=False,
                            skip_group_check=True,
                        )
                nc.tensor.matmul(
                    attn_ps,
                    kt_sb[:, kt * 128 : (kt + 1) * 128],
                    qt_sb[:, qh * qg : (qh + 1) * qg],
                    start=False,
                    stop=True,
                    skip_group_check=True,
                )
                expattn = ework.tile([128, qg], mm_dt, name="expattn")
                nc.scalar.activation(expattn, attn_ps, expf, scale=exp_scale)
                return expattn

            def stage_av(av_ps, kt, expattn):
                for qc in range(n_qc):
                    nc.tensor.matmul(
                        av_ps[qc],
                        expattn[:, qc * 128 : (qc + 1) * 128],
                        vp_sb[:, kt, :],
                        start=(kt == 0),
                        stop=(kt == n_kt - 1),
                    )

            def out_stage(qh, av_ps):
                out_t = outp.tile([128, n_qc, QDIM], F32, name="out_t", tag="ot")
                h = n_qc // 2
                for qc in range(n_qc):
                    recip = outp.tile([128, 1], F32, name="recip", tag="rc")
                    nc.vector.reciprocal_approx_fast(
                        recip, av_ps[qc][:, QDIM : QDIM + 1])
                    if qc % 2 == 0:
                        nc.vector.tensor_scalar_mul(
                            out_t[:, qc, :], av_ps[qc][:, 0:QDIM], recip)
                    else:
                        nc.scalar.activation(
                            out_t[:, qc, :], av_ps[qc][:, 0:QDIM], ident,
                            scale=recip)
                    if qc == h - 1:
                        nc.sync.dma_start(out_r[qh][:, 0:h, :],
                                          out_t[:, 0:h, :])
                nc.sync.dma_start(out_r[qh][:, h:n_qc, :], out_t[:, h:n_qc, :])

            # ---- qh = 0: attention fused with K-prep (one pair ahead) ----
            n_k2 = n_kt // 2
            kprep_t(0)
            kprep_p(0)
            av_ps = [
                avpsum.tile([128, vw], F32, tag=f"av{qc}", name=f"av{qc}")
                for qc in range(n_qc)
            ]
            pend = []
            for kt in range(n_kt):
                k2, r = divmod(kt, 2)
                if r == 0 and k2 + 1 < n_k2:
                    kprep_t(k2 + 1)
                pend.append((kt, stage_a(0, kt)))
                if r == 0 and k2 + 1 < n_k2:
                    kprep_p(k2 + 1)
                if len(pend) > 1:
                    stage_av(av_ps, *pend.pop(0))
            for item in pend:
                stage_av(av_ps, *item)
            out_stage(0, av_ps)

            # ---- remaining q groups: plain pipelined attention ----
            for qh in range(1, n_qg):
                av_ps = [
                    avpsum.tile([128, vw], F32, tag=f"av{qc}", name=f"av{qc}")
                    for qc in range(n_qc)
                ]
                pend = []
                for kt in range(n_kt):
                    pend.append((kt, stage_a(qh, kt)))
                    if len(pend) > lookahead:
                        stage_av(av_ps, *pend.pop(0))
                for item in pend:
                    stage_av(av_ps, *item)
                out_stage(qh, av_ps)


def build_nc(nq, nkeys, q_group=512, mm_dt=F32R, mask_mode=None, repeat=1,
             lookahead=2):
    if mask_mode is None:
        mask_mode = os.environ.get("ADJ_MASK_MODE", "fp8")
    nc = bacc.Bacc("TRN2", target_bir_lowering=False, debug=False,
                   enable_asserts=False)
    io = declare_io(nc, nq, nkeys)
    with tile.TileContext(nc) as tc:
        for _ in range(repeat):
            emit_kernel(tc, nq, nkeys, q_group=q_group, mm_dt=mm_dt,
                        mask_mode=mask_mode, lookahead=lookahead, io=io)
    nc.compile()
    return nc


def make_in_maps(inputs, n_cores=N_CORES):
    """Shard full inputs into per-core input maps."""
    main_feat = np.ascontiguousarray(np.asarray(inputs["main_feat"], dtype=np.float32))
    other_feat = np.ascontiguousarray(np.asarray(inputs["other_feat"], dtype=np.float32))
    fix_feat = np.ascontiguousarray(
        np.asarray(inputs["fix_feat"], dtype=np.float32).reshape(-1, 1)
    )
    mask = np.ascontiguousarray(np.asarray(inputs["mask"])).view(np.uint8)
    wq_ = np.ascontiguousarray(np.asarray(inputs["Wq"], dtype=np.float32))
    bq_ = np.ascontiguousarray(np.asarray(inputs["bq"], dtype=np.float32).reshape(-1, 1))
    wk_ = np.ascontiguousarray(np.asarray(inputs["Wk"], dtype=np.float32))
    bk_ = np.ascontiguousarray(np.asarray(inputs["bk"], dtype=np.float32).reshape(-1, 1))

    n = main_feat.shape[0]
    per = n // n_cores
    in_maps = []
    for c in range(n_cores):
        sl = slice(c * per, (c + 1) * per)
        in_maps.append(
            {
                "main": np.ascontiguousarray(main_feat[sl]),
                "mask": np.ascontiguousarray(mask[sl]),
                "other": other_feat,
                "fix": fix_feat,
                "Wq": wq_,
                "bq": bq_,
                "Wk": wk_,
                "bk": bk_,
            }
        )
    return in_maps


_NC_CACHE = {}


def _get_nc(nq, nkeys):
    key = (nq, nkeys)
    if key not in _NC_CACHE:
        _NC_CACHE[key] = build_nc(nq, nkeys)
    return _NC_CACHE[key]


class _Executor:
    """Cached jit(shard_map) wrapper around the compiled Bass module so
    repeated kernel() calls skip retracing/recompiling."""

    def __init__(self, nc, n_cores=N_CORES):
        import jax
        from jax.sharding import Mesh, PartitionSpec
        from jax.experimental.shard_map import shard_map
        from concourse import bass2jax
        from concourse.bass2jax import _bass_exec_p, install_neuronx_cc_hook

        install_neuronx_cc_hook()
        self.n_cores = n_cores
        partition_name = (
            nc.partition_id_tensor.name if nc.partition_id_tensor else None
        )
        in_names, out_names, out_avals = [], [], []
        for alloc in nc.m.functions[0].allocations:
            if not isinstance(alloc, mybir.MemoryLocationSet):
                continue
            name = alloc.memorylocations[0].name
            if alloc.kind == "ExternalInput":
                if name != partition_name:
                    in_names.append(name)
            elif alloc.kind == "ExternalOutput":
                out_names.append(name)
                out_avals.append(
                    jax.core.ShapedArray(
                        tuple(alloc.tensor_shape), mybir.dt.np(alloc.dtype)
                    )
                )
        self.in_names = list(in_names)
        self.out_names = out_names
        self.out_avals = out_avals
        all_names = in_names + out_names
        if partition_name is not None:
            all_names.append(partition_name)

        def _body(*args):
            operands = list(args)
            if partition_name is not None:
                operands.append(bass2jax.partition_id_tensor())
            return tuple(
                _bass_exec_p.bind(
                    *operands,
                    out_avals=tuple(out_avals),
                    in_names=tuple(all_names),
                    out_names=tuple(out_names),
                    lowering_input_output_aliases=(),
                    sim_require_finite=True,
                    sim_require_nnan=True,
                    nc=nc,
                )
            )

        devices = jax.devices()[:n_cores]
        self.mesh = Mesh(np.asarray(devices), ("core",))
        n_args = len(self.in_names) + len(out_names)
        self.f = jax.jit(
            shard_map(
                _body,
                mesh=self.mesh,
                in_specs=(PartitionSpec("core"),) * n_args,
                out_specs=(PartitionSpec("core"),) * len(out_names),
                check_rep=False,
            ),
            keep_unused=True,
        )

    def run(self, in_maps):
        concat_in = [
            np.concatenate([m[nm] for m in in_maps], axis=0)
            for nm in self.in_names
        ]
        concat_zeros = [
            np.zeros((self.n_cores * a.shape[0], *a.shape[1:]), a.dtype)
            for a in self.out_avals
        ]
        r = self.f(*concat_in, *concat_zeros)
        return np.asarray(r[0])


_EXEC_CACHE = {}


def _get_executor(nq, nkeys):
    key = (nq, nkeys)
    if key not in _EXEC_CACHE:
        _EXEC_CACHE[key] = _Executor(_get_nc(nq, nkeys))
    return _EXEC_CACHE[key]


def kernel(**inputs) -> np.ndarray:
    n = np.asarray(inputs["main_feat"]).shape[0]
    nkeys = np.asarray(inputs["other_feat"]).shape[0]
    in_maps = make_in_maps(inputs, N_CORES)
    try:
        ex = _get_executor(n // N_CORES, nkeys)
        return ex.run(in_maps)
    except Exception:
        nc = _get_nc(n // N_CORES, nkeys)
        res = run_bass_kernel_spmd(nc, in_maps, core_ids=list(range(N_CORES)))
        return np.concatenate(
            [res.results[c]["out"] for c in range(N_CORES)], axis=0
        )


# revision 21
# speedup vs baseline: 2.8685x; 1.9687x over previous
"""Trainium2 Bass kernel for AdjAttenAgger-style masked cross-attention.

Computes, for full inputs:
    Q = main_feat @ Wq.T + bq              # [N, MID]
    K = other_feat @ Wk.T + bk             # [M, MID]
    attn = softmax(where(mask, -BIG, Q K^T / sqrt(MID)), axis=-1)
    out  = attn @ (fix_feat[:, None] * other_feat)          # [N, KDIM]

Sharding: rows of main_feat/mask (the N query axis) are split across 8
NeuronCores; other_feat/fix_feat/weights are replicated. No collectives.

Per-core dataflow (all layouts chosen so no large tensor is ever
transposed outside the PE array):
  - QT [MID, nq] and KT [MID, nk] are built dim-major via PE-transposed
    input tiles, so the QK^T matmul directly produces attnT [k, q] slabs.
  - The boolean mask (q-major in DRAM, only efficiently loadable q-major)
    is applied *by the PE*: accumulating matmuls with the q-major mask
    tile as the stationary operand and a scaled diagonal as the moving
    operand add -BIG * mask^T into the attnT PSUM tile.
  - ACT computes exp((attnT - BIG*mask)/sqrt(MID)) PSUM->SBUF; no row-max
    subtraction is needed (logits are O(1); masked entries underflow to 0).
  - V' = [fix*other | 1] has an extra ones column, so the attn@V' matmul
    also produces the softmax denominators; a per-row divide finishes the
    softmax normalization on the [nq, 256] output only.

Scheduling: the prologue batches the four PE transposes of each 256-row
input chunk into a single PSUM bank evacuated by one DVE copy, and the
main loop emits the mask/QK matmuls of k-tile kt+2 before the AV matmuls
of k-tile kt, so the PE never stalls on the exp activation.
"""

import math
import os

import numpy as np

import concourse.bass as bass
from concourse import bacc
import concourse.mybir as mybir
import concourse.tile as tile
from concourse.bass_utils import run_bass_kernel_spmd

F32 = mybir.dt.float32
F32R = mybir.dt.float32r
BF16 = mybir.dt.bfloat16
U8 = mybir.dt.uint8
F8E4 = mybir.dt.float8e4

N_CORES = 8
QDIM = 256       # main/other feature dim
MID = 128
NEG_BIG = -float(2 ** 41)  # additive pre-scale mask value; exp() underflows to 0
F8_SUB = 2.0 ** -9         # value of byte 0x01 reinterpreted as float8e4 (e4m3)
F8_MAX = 240.0             # fp8e4 (IEEE e4m3) max normal
EPS_DR = 2.0 ** -12        # Q prescale for fp8x8 DoubleRow mask (product -0.875)


def _diag(nc, ap, fill):
    """ap[i, j] = fill if i == j else 0."""
    nc.gpsimd.memset(ap, 0.0)
    nc.gpsimd.affine_select(
        out=ap, in_=ap,
        compare_op=mybir.AluOpType.not_equal,
        fill=fill, base=0,
        pattern=[[-1, ap.shape[1]]],
        channel_multiplier=1,
    )


def declare_io(nc, nq, nkeys):
    return {
        "main": nc.dram_tensor("main", [nq, QDIM], F32, kind="ExternalInput").ap(),
        "mask": nc.dram_tensor("mask", [nq, nkeys], U8, kind="ExternalInput").ap(),
        "other": nc.dram_tensor("other", [nkeys, QDIM], F32, kind="ExternalInput").ap(),
        "fix": nc.dram_tensor("fix", [nkeys, 1], F32, kind="ExternalInput").ap(),
        "Wq": nc.dram_tensor("Wq", [MID, QDIM], F32, kind="ExternalInput").ap(),
        "bq": nc.dram_tensor("bq", [MID, 1], F32, kind="ExternalInput").ap(),
        "Wk": nc.dram_tensor("Wk", [MID, QDIM], F32, kind="ExternalInput").ap(),
        "bk": nc.dram_tensor("bk", [MID, 1], F32, kind="ExternalInput").ap(),
        "out": nc.dram_tensor("out", [nq, QDIM], F32, kind="ExternalOutput").ap(),
    }


def emit_kernel(tc, nq, nkeys, q_group=512, mm_dt=F32R, mask_mode="fp8",
                lookahead=2, io=None):
    """Emit the per-core program. nq = queries this core, nkeys = all keys."""
    nc = tc.nc
    n_qt = nq // 128          # query 128-tiles
    n_kt = nkeys // 128       # key 128-tiles
    qg = min(q_group, nq)     # q columns per PSUM slab
    n_qg = nq // qg
    n_qc = qg // 128          # 128-chunks per q group
    inv_sqrt_mid = 1.0 / math.sqrt(MID)
    vw = QDIM + 2             # V' width: 256 dims + ones col + pad (even for f32r)

    if io is None:
        io = declare_io(nc, nq, nkeys)
    main, maskd, other, fix = io["main"], io["mask"], io["other"], io["fix"]
    wq, bq, wk, bk, out = io["Wq"], io["bq"], io["Wk"], io["bk"], io["out"]

    # mask viewed as [qg-group, kt-group, partition(q), qc-chunk, k]
    KTG = min(16, n_kt)    # key tiles per mask DMA (2KB contiguous chunks)
    mask_rg = maskd.rearrange(
        "(qh qc p) (ktg k) -> qh ktg p qc k", qc=n_qc, p=128, k=KTG * 128
    )
    n_qc2 = max(1, qg // 256)  # 256-row chunks for DoubleRow mask MMs
    mask_dr = maskd.rearrange(
        "(qh qc2 p j) (ktg k) -> qh ktg p qc2 j k",
        qc2=n_qc2, p=qg // (2 * n_qc2), j=2, k=KTG * 128,
    )

    ident = mybir.ActivationFunctionType.Identity
    expf = mybir.ActivationFunctionType.Exp

    with (
        tc.tile_pool(name="const", bufs=1) as constp,
        tc.tile_pool(name="big", bufs=1) as bigp,
        tc.tile_pool(name="mwork", bufs=1) as mwork,
    ):
        # ---- constants (ident/diag first: no DMA dependency) ----
        ident_f32 = constp.tile([128, 128], F32)
        _diag(nc, ident_f32, 1.0)
        if mm_dt == F32:
            ident_t = ident_f32
        else:
            ident_t = constp.tile([128, 128], mm_dt)
            nc.vector.tensor_copy(ident_t, ident_f32)
        if mask_mode == "fp8":
            diag_mm = constp.tile([128, 128], BF16)
            _diag(nc, diag_mm, NEG_BIG * F8_SUB)  # f8 byte 0x01 -> 2^-9
        else:  # fp8dr
            diag_dr = constp.tile([128, 2, 256], F8E4)
            nc.gpsimd.memset(diag_dr, 0.0)
            # fill where 2*ki + j - q' == 0
            nc.gpsimd.affine_select(
                out=diag_dr, in_=diag_dr,
                compare_op=mybir.AluOpType.not_equal,
                fill=-F8_MAX, base=0,
                pattern=[[1, 2], [-1, 256]],
                channel_multiplier=2,
            )

        wq_s = constp.tile([MID, QDIM], mm_dt)
        wk_s = constp.tile([MID, QDIM], mm_dt)
        bq_s = constp.tile([MID, 1], F32)
        nc.sync.dma_start(bq_s, bq)
        if mask_mode == "fp8dr":
            bq_eps = constp.tile([MID, 1], F32)
            nc.vector.tensor_scalar_mul(bq_eps, bq_s, EPS_DR)
            q_bias, q_scale = bq_eps, EPS_DR
            exp_scale = inv_sqrt_mid / EPS_DR
        else:
            q_bias, q_scale = bq_s, 1.0
            exp_scale = inv_sqrt_mid
        bk_s = constp.tile([MID, 1], F32)
        nc.sync.dma_start(bk_s, bk)

        # ---- persistent big tensors ----
        kt_sb = bigp.tile([MID, nkeys], mm_dt)      # K^T, dim-major
        qt_sb = bigp.tile([MID, nq], mm_dt)         # Q^T, dim-major
        vp_sb = bigp.tile([128, n_kt, vw], mm_dt)   # V' tiles, token-major
        nc.scalar.activation(vp_sb[:, :, QDIM : QDIM + 2], vp_sb[:, :, 0:2],
                             mybir.ActivationFunctionType.Copy,
                             bias=1.0, scale=0.0)

        # ---- mask prefetch machinery (pool stays open for the main loop) ----
        n_ktg = n_kt // KTG
        mask_bufs = 3
        mask_tiles = {}

        def fetch_mask(qh, ktg):
            if mask_mode == "fp8dr":
                mg = mwork.tile([128, n_qc2, 2, KTG * 128], F8E4,
                                name="mask_gdr", tag="mg", bufs=mask_bufs)
                for qc2 in range(n_qc2):
                    nc.sync.dma_start(
                        mg[:, qc2], mask_dr[qh, ktg][:, qc2].bitcast(F8E4))
            else:
                mg = mwork.tile([128, n_qc, KTG * 128], F8E4,
                                name="mask_gf8", tag="mg", bufs=mask_bufs)
                nc.sync.dma_start(mg, mask_rg[qh, ktg].bitcast(F8E4))
            mask_tiles[(qh, ktg)] = mg

        wqt_s = constp.tile([128, 2, MID], mm_dt)
        wkt_s = constp.tile([128, 2, MID], mm_dt)
        fix_s = constp.tile([128, n_kt], F32)
        out_r = out.rearrange("(qh qc p) d -> qh p qc d", qc=n_qc, p=128)

        with (
            tc.tile_pool(name="prologue", bufs=4) as prop,
            tc.tile_pool(name="pps", bufs=1, space="PSUM") as pps,
            tc.tile_pool(name="avpsum", bufs=1, space="PSUM") as avpsum,
            tc.tile_pool(name="ework", bufs=4) as ework,
            tc.tile_pool(name="outp", bufs=2) as outp,
        ):
            # One shared 3-deep rotation of 2KB PSUM banks serves the
            # transpose staging tiles AND the attention slabs, so K-prep can
            # interleave with the qh=0 attention loop inside 8 PSUM banks
            # (3 shared + 1 proj + 4 AV accumulators).
            def ps_tile(shape, dtype, name):
                return pps.tile(shape, dtype, name=name, tag="ps", bufs=3)

            def mps_tile(shape, name):
                return pps.tile(shape, F32, name=name, tag="mps", bufs=1)

            warm_ps = ps_tile([128, 4, 128], mm_dt, "warm_ps")
            for w in range(4):
                nc.tensor.transpose(warm_ps[:, w, :], ident_t, ident_t)

            # ---- Q^T = Wq @ main^T + bq  (pairs of 128-tiles: 256 moving) ----
            # main tiles DMA'd ahead of the weights so the PE's first
            # transposes start as early as possible
            main_ts = []
            for t2 in range(n_qt // 2):
                main_t = prop.tile([128, 2, QDIM], mm_dt, name="main_t", tag="in")
                nc.sync.dma_start(
                    main_t, main[t2 * 256 : (t2 + 1) * 256, :]
                    .rearrange("(a p) d -> p a d", p=128).bitcast(mm_dt)
                )
                main_ts.append(main_t)
                if t2 == 0:
                    nc.sync.dma_start(wq_s, wq.bitcast(mm_dt))
                    nc.sync.dma_start(wk_s, wk.bitcast(mm_dt))
            for t2 in range(n_qt // 2):
                main_t = main_ts[t2]
                tp_ps = ps_tile([128, 4, 128], mm_dt, "tp_ps")
                for h in range(2):
                    for a in range(2):
                        nc.tensor.transpose(
                            tp_ps[:, 2 * h + a, :],
                            main_t[:, a, h * 128 : (h + 1) * 128], ident_t,
                        )
                maint_s = prop.tile([128, 4, 128], mm_dt, name="maint_s", tag="tr")
                nc.vector.tensor_copy(maint_s, tp_ps)
                if t2 == 0:
                    # WqT / WkT transposes slot in behind the first Q tile
                    wt_ps = ps_tile([128, 4, MID], mm_dt, "wt_ps")
                    for h in range(2):
                        nc.tensor.transpose(
                            wt_ps[:, h, :],
                            wq_s[:, h * 128 : (h + 1) * 128], ident_t)
                        nc.tensor.transpose(
                            wt_ps[:, 2 + h, :],
                            wk_s[:, h * 128 : (h + 1) * 128], ident_t)
                    nc.vector.tensor_copy(wqt_s, wt_ps[:, 0:2, :])
                    nc.vector.tensor_copy(wkt_s, wt_ps[:, 2:4, :])
                q_ps = mps_tile([MID, 256], "q_ps")
                for h in range(2):
                    nc.tensor.matmul(
                        q_ps,
                        wqt_s[:, h, :],
                        maint_s[:, 2 * h : 2 * h + 2, :],
                        start=(h == 0),
                        stop=(h == 1),
                    )
                nc.scalar.activation(
                    qt_sb[:, t2 * 256 : (t2 + 1) * 256], q_ps, ident,
                    bias=q_bias, scale=q_scale,
                )

            # fix loaded with one contiguous DMA [tile, 128] then PE-transposed
            # to the per-partition layout fix_s[p, kt] = fix[kt*128 + p]
            assert n_kt <= 128
            fix_tT = prop.tile([n_kt, 128], F32, name="fix_tT", tag="fT")
            nc.sync.dma_start(
                fix_tT, fix.rearrange("(t p) d -> t (p d)", p=128)
            )
            ft_ps = mps_tile([128, n_kt], "ft_ps")
            nc.tensor.transpose(ft_ps, fix_tT, ident_f32[0:n_kt, 0:n_kt])
            nc.vector.tensor_copy(fix_s, ft_ps)

            fetch_mask(0, 0)

            # ---- K-prep: K^T = Wk @ other^T + bk ; V' = [fix*other | 1] ----
            # other rows are DMA'd straight into vp_sb; the PE transposes
            # read the raw rows from there, after which gpsimd scales them
            # by fix in place (ordered by the tile dep tracker). Split in two
            # halves so the projection matmul (which needs the DVE-evacuated
            # transposes) is emitted an attention-stage later than the
            # transposes themselves.
            kprep_st = {}

            def kprep_t(k2):
                other_t = vp_sb[:, 2 * k2 : 2 * k2 + 2, 0:QDIM]
                nc.sync.dma_start(
                    other_t, other[k2 * 256 : (k2 + 1) * 256, :]
                    .rearrange("(a p) d -> p a d", p=128).bitcast(mm_dt)
                )
                to_ps = ps_tile([128, 4, 128], mm_dt, "to_ps")
                for h in range(2):
                    for a in range(2):
                        nc.tensor.transpose(
                            to_ps[:, 2 * h + a, :],
                            other_t[:, a, h * 128 : (h + 1) * 128], ident_t,
                        )
                for a in range(2):
                    kt = 2 * k2 + a
                    nc.gpsimd.tensor_scalar_mul(
                        vp_sb[:, kt, 0:QDIM],
                        other_t[:, a, :],
                        fix_s[:, kt : kt + 1],
                    )
                ot_s = prop.tile([128, 4, 128], mm_dt, name="ot_s", tag="tr")
                nc.vector.tensor_copy(ot_s, to_ps)
                kprep_st[k2] = ot_s

            def kprep_p(k2):
                ot_s = kprep_st.pop(k2)
                k_ps = mps_tile([MID, 256], "k_ps")
                for h in range(2):
                    nc.tensor.matmul(
                        k_ps,
                        wkt_s[:, h, :],
                        ot_s[:, 2 * h : 2 * h + 2, :],
                        start=(h == 0),
                        stop=(h == 1),
                    )
                nc.scalar.activation(
                    kt_sb[:, k2 * 256 : (k2 + 1) * 256], k_ps, ident, bias=bk_s
                )

            def stage_a(qh, kt):
                """Mask + QK matmuls into a fresh PSUM slab, then exp."""
                ktg, kti = divmod(kt, KTG)
                if (qh, ktg) not in mask_tiles:
                    fetch_mask(qh, ktg)
                if kti == 0:
                    nqh, ngt = (qh, ktg + 1) if ktg + 1 < n_ktg else (qh + 1, 0)
                    if nqh < n_qg and (nqh, ngt) not in mask_tiles:
                        fetch_mask(nqh, ngt)
                mg = mask_tiles[(qh, ktg)]
                attn_ps = ps_tile([128, qg], F32, "attn_ps")
                if mask_mode == "fp8dr":
                    for qc2 in range(n_qc2):
                        nc.tensor.matmul(
                            attn_ps[:, qc2 * 256 : (qc2 + 1) * 256],
                            mg[:, qc2, :, kti * 128 : (kti + 1) * 128],
                            diag_dr,
                            start=(qc2 == 0),
                            stop=False,
                            perf_mode=mybir.MatmulPerfMode.DoubleRow,
                            skip_group_check=True,
                        )
                else:
                    for qc in range(n_qc):
                        nc.tensor.matmul(
                            attn_ps[:, qc * 128 : (qc + 1) * 128],
                            mg[:, qc, kti * 128 : (kti + 1) * 128],
                            diag_mm,
                            start=(qc == 0),
                            stop=False,
                            skip_group_check=True,
                        )
                nc.tensor.matmul(
                    attn_ps,
                    kt_sb[:, kt * 128 : (kt + 1) * 128],
                    qt_sb[:, qh * qg : (qh + 1) * qg],
                    start=False,
                    stop=True,
                    skip_group_check=True,
                )
                expattn = ework.tile([128, qg], mm_dt, name="expattn")
                nc.scalar.activation(expattn, attn_ps, expf, scale=exp_scale)
                return expattn

            def stage_av(av_ps, kt, expattn):
                for qc in range(n_qc):
                    nc.tensor.matmul(
                        av_ps[qc],
                        expattn[:, qc * 128 : (qc + 1) * 128],
                        vp_sb[:, kt, :],
                        start=(kt == 0),
                        stop=(kt == n_kt - 1),
                    )

            def out_stage(qh, av_ps):
                out_t = outp.tile([128, n_qc, QDIM], F32, name="out_t", tag="ot")
                h = n_qc // 2
                for qc in range(n_qc):
                    recip = outp.tile([128, 1], F32, name="recip", tag="rc")
                    nc.vector.reciprocal_approx_fast(
                        recip, av_ps[qc][:, QDIM : QDIM + 1])
                    if qc % 2 == 0:
                        nc.vector.tensor_scalar_mul(
                            out_t[:, qc, :], av_ps[qc][:, 0:QDIM], recip)
                    else:
                        nc.scalar.activation(
                            out_t[:, qc, :], av_ps[qc][:, 0:QDIM], ident,
                            scale=recip)
                    if qc == h - 1:
                        nc.sync.dma_start(out_r[qh][:, 0:h, :],
                                          out_t[:, 0:h, :])
                nc.sync.dma_start(out_r[qh][:, h:n_qc, :], out_t[:, h:n_qc, :])

            # ---- qh = 0: attention fused with K-prep (one pair ahead) ----
            n_k2 = n_kt // 2
            kprep_t(0)
            kprep_p(0)
            av_ps = [
                avpsum.tile([128, vw], F32, tag=f"av{qc}", name=f"av{qc}")
                for qc in range(n_qc)
            ]
            pend = []
            for kt in range(n_kt):
                k2, r = divmod(kt, 2)
                if r == 0 and k2 + 1 < n_k2:
                    kprep_t(k2 + 1)
                pend.append((kt, stage_a(0, kt)))
                if r == 0 and k2 + 1 < n_k2:
                    kprep_p(k2 + 1)
                if len(pend) > 1:
                    stage_av(av_ps, *pend.pop(0))
            for item in pend:
                stage_av(av_ps, *item)
            out_stage(0, av_ps)

            # ---- remaining q groups: plain pipelined attention ----
            for qh in range(1, n_qg):
                av_ps = [
                    avpsum.tile([128, vw], F32, tag=f"av{qc}", name=f"av{qc}")
                    for qc in range(n_qc)
                ]
                pend = []
                for kt in range(n_kt):
                    pend.append((kt, stage_a(qh, kt)))
                    if len(pend) > lookahead:
                        stage_av(av_ps, *pend.pop(0))
                for item in pend:
                    stage_av(av_ps, *item)
                out_stage(qh, av_ps)


def build_nc(nq, nkeys, q_group=512, mm_dt=F32R, mask_mode=None, repeat=1,
             lookahead=2):
    if mask_mode is None:
        mask_mode = os.environ.get("ADJ_MASK_MODE", "fp8dr")
    nc = bacc.Bacc("TRN2", target_bir_lowering=False, debug=False,
                   enable_asserts=False)
    io = declare_io(nc, nq, nkeys)
    with tile.TileContext(nc) as tc:
        for _ in range(repeat):
            emit_kernel(tc, nq, nkeys, q_group=q_group, mm_dt=mm_dt,
                        mask_mode=mask_mode, lookahead=lookahead, io=io)
    nc.compile()
    return nc


def make_in_maps(inputs, n_cores=N_CORES):
    """Shard full inputs into per-core input maps."""
    main_feat = np.ascontiguousarray(np.asarray(inputs["main_feat"], dtype=np.float32))
    other_feat = np.ascontiguousarray(np.asarray(inputs["other_feat"], dtype=np.float32))
    fix_feat = np.ascontiguousarray(
        np.asarray(inputs["fix_feat"], dtype=np.float32).reshape(-1, 1)
    )
    mask = np.ascontiguousarray(np.asarray(inputs["mask"])).view(np.uint8)
    wq_ = np.ascontiguousarray(np.asarray(inputs["Wq"], dtype=np.float32))
    bq_ = np.ascontiguousarray(np.asarray(inputs["bq"], dtype=np.float32).reshape(-1, 1))
    wk_ = np.ascontiguousarray(np.asarray(inputs["Wk"], dtype=np.float32))
    bk_ = np.ascontiguousarray(np.asarray(inputs["bk"], dtype=np.float32).reshape(-1, 1))

    n = main_feat.shape[0]
    per = n // n_cores
    in_maps = []
    for c in range(n_cores):
        sl = slice(c * per, (c + 1) * per)
        in_maps.append(
            {
                "main": np.ascontiguousarray(main_feat[sl]),
                "mask": np.ascontiguousarray(mask[sl]),
                "other": other_feat,
                "fix": fix_feat,
                "Wq": wq_,
                "bq": bq_,
                "Wk": wk_,
                "bk": bk_,
            }
        )
    return in_maps


_NC_CACHE = {}


def _get_nc(nq, nkeys):
    key = (nq, nkeys)
    if key not in _NC_CACHE:
        _NC_CACHE[key] = build_nc(nq, nkeys)
    return _NC_CACHE[key]


class _Executor:
    """Cached jit(shard_map) wrapper around the compiled Bass module so
    repeated kernel() calls skip retracing/recompiling."""

    def __init__(self, nc, n_cores=N_CORES):
        import jax
        from jax.sharding import Mesh, PartitionSpec
        from jax.experimental.shard_map import shard_map
        from concourse import bass2jax
        from concourse.bass2jax import _bass_exec_p, install_neuronx_cc_hook

        install_neuronx_cc_hook()
        self.n_cores = n_cores
        partition_name = (
            nc.partition_id_tensor.name if nc.partition_id_tensor else None
        )
        in_names, out_names, out_avals = [], [], []
        for alloc in nc.m.functions[0].allocations:
            if not isinstance(alloc, mybir.MemoryLocationSet):
                continue
            name = alloc.memorylocations[0].name
            if alloc.kind == "ExternalInput":
                if name != partition_name:
                    in_names.append(name)
            elif alloc.kind == "ExternalOutput":
                out_names.append(name)
                out_avals.append(
                    jax.core.ShapedArray(
                        tuple(alloc.tensor_shape), mybir.dt.np(alloc.dtype)
                    )
                )
        self.in_names = list(in_names)
        self.out_names = out_names
        self.out_avals = out_avals
        all_names = in_names + out_names
        if partition_name is not None:
            all_names.append(partition_name)

        def _body(*args):
            operands = list(args)
            if partition_name is not None:
                operands.append(bass2jax.partition_id_tensor())
            return tuple(
                _bass_exec_p.bind(
                    *operands,
                    out_avals=tuple(out_avals),
                    in_names=tuple(all_names),
                    out_names=tuple(out_names),
                    lowering_input_output_aliases=(),
                    sim_require_finite=True,
                    sim_require_nnan=True,
                    nc=nc,
                )
            )

        devices = jax.devices()[:n_cores]
        self.mesh = Mesh(np.asarray(devices), ("core",))
        n_args = len(self.in_names) + len(out_names)
        self.f = jax.jit(
            shard_map(
                _body,
                mesh=self.mesh,
                in_specs=(PartitionSpec("core"),) * n_args,
                out_specs=(PartitionSpec("core"),) * len(out_names),
                check_rep=False,
            ),
            keep_unused=True,
        )

    def run(self, in_maps):
        concat_in = [
            np.concatenate([m[nm] for m in in_maps], axis=0)
            for nm in self.in_names
        ]
        concat_zeros = [
            np.zeros((self.n_cores * a.shape[0], *a.shape[1:]), a.dtype)
            for a in self.out_avals
        ]
        r = self.f(*concat_in, *concat_zeros)
        return np.asarray(r[0])


_EXEC_CACHE = {}


def _get_executor(nq, nkeys):
    key = (nq, nkeys)
    if key not in _EXEC_CACHE:
        _EXEC_CACHE[key] = _Executor(_get_nc(nq, nkeys))
    return _EXEC_CACHE[key]


def kernel(**inputs) -> np.ndarray:
    n = np.asarray(inputs["main_feat"]).shape[0]
    nkeys = np.asarray(inputs["other_feat"]).shape[0]
    in_maps = make_in_maps(inputs, N_CORES)
    try:
        ex = _get_executor(n // N_CORES, nkeys)
        return ex.run(in_maps)
    except Exception:
        nc = _get_nc(n // N_CORES, nkeys)
        res = run_bass_kernel_spmd(nc, in_maps, core_ids=list(range(N_CORES)))
        return np.concatenate(
            [res.results[c]["out"] for c in range(N_CORES)], axis=0
        )
